# revision 25
# baseline (speedup 1.0000x reference)
"""Trainium2 Bass kernel for a 2-layer dual-direction gated GCN (DGGCN).

Contract: kernel(**inputs) takes the FULL unsharded inputs (as produced by
setup_inputs) and returns the FULL [N, D] float32 output.

Strategy (8 NeuronCores, node partition):
  - Nodes are remapped into a chunk-major padded slot space: the node axis is
    split into ag_chunks regions; within a region, cores' tiles are laid out
    contiguously per core so a chunked AllGather writes each region with one
    contiguous collective. Self-loops are explicit (v, v) edges. Edges are
    bucketed by destination tile (forward) / source tile (reverse) and split
    per tile into lo/hi halves of the slot space (region boundary) so gather
    indices fit dma_gather's int16 format. Uniform B_LO/B_HI block counts per
    tile keep the program SPMD.
  - Per layer each core computes h @ W for OWN tiles only, writes its slab,
    and a chunked AllGather replicates the bf16 message table to all cores.
    dinv[src] is NOT folded into the table; instead it is folded into the
    one-hot scatter matrices, so one table serves both edge directions.
  - Aggregation per own dst tile: batched dma_gather of table rows. All
    gathers are issued as prepare_only descriptor-generation (GPSIMD) one
    batch ahead and fired with trigger_dma, so SWDGE generation overlaps
    compute/DMA. Scatter matrices are built on-chip with a fused
    tensor_scalar (iota == dst_code) * dinv_src, then PE matmuls accumulate
    segment sums in PSUM; relu(x)*dinv_dst is fused on ACT.
"""

import os
import sys

sys.path.insert(0, "/opt/trn_rl_repo")

import numpy as np

import concourse.bacc as bacc
import concourse.bass as bass
import concourse.tile as tile
from concourse import mybir
from concourse.bass_utils import run_bass_kernel_spmd
from concourse.masks import make_identity

F32 = mybir.dt.float32
BF16 = mybir.dt.bfloat16
I32 = mybir.dt.int32
I16 = mybir.dt.int16

W_CORES = 8
D = 128
PAD_DST = 200.0  # sentinel local-dst id (never matches iota 0..127)
AG_CHUNKS = 2


# ---------------------------------------------------------------------------
# host-side graph preprocessing (index bucketing / sharding metadata only)
# ---------------------------------------------------------------------------


def _chunk_sizes(t_own):
    sizes = []
    base = 0
    for j in range(AG_CHUNKS):
        n = (t_own - base) // (AG_CHUNKS - j)
        if n > 0:
            sizes.append(n)
            base += n
    return sizes


def _pack16(flat):
    """Pack an int array [n] into dma_gather's [128, n//16] int16 layout:
    index i lives at partition i%16, column i//16, replicated to 8 stripes."""
    n = flat.shape[0]
    assert n % 16 == 0
    return np.tile(flat.reshape(n // 16, 16).T, (8, 1))


def host_prepare(x, edge_index, n_real):
    w = W_CORES
    assert n_real % w == 0
    sh_real = n_real // w
    t_own = (sh_real + 127) // 128
    sh_pad = t_own * 128
    t_all = w * t_own
    np_pad = t_all * 128
    sizes = _chunk_sizes(t_own)
    # chunk-major region starts (rows) and per-core tile starts within chunks
    reg_rows = [w * n * 128 for n in sizes]
    reg_off = np.concatenate([[0], np.cumsum(reg_rows)]).astype(np.int64)
    r_split = int(reg_off[1])  # lo/hi gather split at region-0 boundary
    assert r_split <= 32767 + 1 and (np_pad - r_split) <= 32767 + 1
    chunk_of_tile = np.concatenate(
        [np.full(n, j, np.int64) for j, n in enumerate(sizes)]
    )
    tile_in_chunk = np.concatenate([np.arange(n) for n in sizes])
    c0_of_chunk = np.concatenate([[0], np.cumsum(sizes)]).astype(np.int64)

    sizes_arr = np.asarray(sizes, np.int64)

    def slot_of(v):
        c = v // sh_real
        l = v % sh_real
        t = l // 128
        off = l % 128
        j = chunk_of_tile[t]
        return reg_off[j] + (c * sizes_arr[j] + tile_in_chunk[t]) * 128 + off

    src = np.asarray(edge_index[0], dtype=np.int64)
    dst = np.asarray(edge_index[1], dtype=np.int64)
    ss = slot_of(src)
    ds = slot_of(dst)
    selfs = slot_of(np.arange(n_real, dtype=np.int64))

    # degrees (+1 self-loop) in slot space; dinv on host
    deg_f = np.ones(np_pad, np.float32)
    deg_r = np.ones(np_pad, np.float32)
    np.add.at(deg_f, ds, 1.0)
    np.add.at(deg_r, ss, 1.0)
    dinv_f = (1.0 / np.sqrt(deg_f)).astype(np.float32)
    dinv_r = (1.0 / np.sqrt(deg_r)).astype(np.float32)

    # global tile id of a slot (for bucketing): region-aware
    def tile_of_slot(s):
        out = np.empty(s.shape, np.int64)
        for j in range(len(sizes)):
            m = (s >= reg_off[j]) & (s < reg_off[j + 1])
            rel = (s[m] - reg_off[j]) // 128  # core-major within region
            c = rel // sizes[j]
            t = rel % sizes[j]
            out[m] = c * t_own + c0_of_chunk[j] + t
        return out

    # self-loops as explicit edges
    agg_f = np.concatenate([ds, selfs])
    gat_f = np.concatenate([ss, selfs])
    agg_r = np.concatenate([ss, selfs])
    gat_r = np.concatenate([ds, selfs])

    def bucket(agg_slot, gather_slot):
        tile_id = tile_of_slot(agg_slot)
        hi = (gather_slot >= r_split).astype(np.int64)
        order = np.lexsort((hi, tile_id))
        t_s = tile_id[order]
        g_s = gather_slot[order]
        h_s = hi[order]
        loc_s = (agg_slot[order] % 128).astype(np.float32)
        n_lo = np.bincount(t_s[h_s == 0], minlength=t_all)
        n_hi = np.bincount(t_s[h_s == 1], minlength=t_all)
        return t_s, g_s, h_s, loc_s, n_lo, n_hi

    bf = bucket(agg_f, gat_f)
    br = bucket(agg_r, gat_r)
    b_lo = int(max(bf[4].max(), br[4].max()) + 127) // 128
    b_hi = int(max(bf[5].max(), br[5].max()) + 127) // 128
    bt = b_lo + b_hi

    def build_tables(t_s, g_s, h_s, loc_s, n_lo, n_hi):
        idx_lo = np.zeros((t_all, b_lo * 128), np.int16)
        idx_hi = np.zeros((t_all, b_hi * 128), np.int16)
        dst_t = np.full((t_all, bt * 128), PAD_DST, np.float32)
        n_edges = len(t_s)
        tile_starts = np.zeros(t_all + 1, np.int64)
        np.cumsum(n_lo + n_hi, out=tile_starts[1:])
        pos_in_tile = np.arange(n_edges) - tile_starts[t_s]
        pos_lo = pos_in_tile
        pos_hi = pos_in_tile - n_lo[t_s]
        mlo = h_s == 0
        mhi = h_s == 1
        idx_lo[t_s[mlo], pos_lo[mlo]] = g_s[mlo].astype(np.int16)
        idx_hi[t_s[mhi], pos_hi[mhi]] = (g_s[mhi] - r_split).astype(np.int16)
        dst_t[t_s[mlo], pos_lo[mlo]] = loc_s[mlo]
        dst_t[t_s[mhi], b_lo * 128 + pos_hi[mhi]] = loc_s[mhi]
        return idx_lo, idx_hi, dst_t

    tbl_f = build_tables(*bf)
    tbl_r = build_tables(*br)

    k = min(3, t_own)
    nb = (t_own + k - 1) // k

    def per_core(idx_lo, idx_hi, dst_t):
        import ml_dtypes
        idxlo_l, idxhi_l, dst_l = [], [], []
        for c in range(w):
            sl = slice(c * t_own, (c + 1) * t_own)
            ilo = idx_lo[sl]
            ihi = idx_hi[sl]
            plo = np.concatenate(
                [_pack16(ilo[g * k : min((g + 1) * k, t_own)].reshape(-1)) for g in range(nb)],
                axis=1,
            )
            phi = np.concatenate(
                [_pack16(ihi[g * k : min((g + 1) * k, t_own)].reshape(-1)) for g in range(nb)],
                axis=1,
            )

            def t_pack(a):
                return np.ascontiguousarray(
                    a.reshape(t_own, bt, 128).transpose(2, 0, 1).reshape(128, t_own * bt)
                ).astype(ml_dtypes.bfloat16)

            idxlo_l.append(np.ascontiguousarray(plo))
            idxhi_l.append(np.ascontiguousarray(phi))
            dst_l.append(t_pack(dst_t[sl]))
        return idxlo_l, idxhi_l, dst_l

    ilo_f, ihi_f, dst_f = per_core(*tbl_f)
    ilo_r, ihi_r, dst_r = per_core(*tbl_r)

    # own-tile dinv columns [128, t_own] per core per direction
    def own_dinv(dinv):
        cols = []
        for c in range(w):
            rows = np.empty(sh_pad, np.float32)
            for t in range(t_own):
                j = chunk_of_tile[t]
                r0 = reg_off[j] + (c * sizes[j] + tile_in_chunk[t]) * 128
                rows[t * 128 : (t + 1) * 128] = dinv[r0 : r0 + 128]
            cols.append(np.ascontiguousarray(rows.reshape(t_own, 128).T))
        return cols

    dof = own_dinv(dinv_f)
    dor = own_dinv(dinv_r)

    meta = dict(
        sh_real=sh_real, sh_pad=sh_pad, t_own=t_own, t_all=t_all, np_pad=np_pad,
        b_lo=b_lo, b_hi=b_hi, k=k, nb=nb, sizes=sizes, r_split=r_split,
        slot_of=slot_of,
    )
    tables = dict(
        ilo_f=ilo_f, ihi_f=ihi_f, dst_f=dst_f,
        ilo_r=ilo_r, ihi_r=ihi_r, dst_r=dst_r,
        dof=dof, dor=dor,
        dallf=np.ascontiguousarray(dinv_f.reshape(t_all, 128).T),
        dallr=np.ascontiguousarray(dinv_r.reshape(t_all, 128).T),
    )
    return meta, tables


# ---------------------------------------------------------------------------
# device program
# ---------------------------------------------------------------------------


def build_program(t_own, b_lo, b_hi, k, add_bc1=False, add_bc2=False):
    w = W_CORES
    t_all = w * t_own
    np_pad = t_all * 128
    sh_pad = t_own * 128
    bt = b_lo + b_hi
    nb = (t_own + k - 1) // k
    kg_of = [min(k, t_own - g * k) for g in range(nb)]
    lo_off = [sum(kg_of[:g]) * b_lo * 8 for g in range(nb + 1)]
    hi_off = [sum(kg_of[:g]) * b_hi * 8 for g in range(nb + 1)]
    lo_cols = lo_off[-1]
    hi_cols = hi_off[-1]
    sizes = _chunk_sizes(t_own)
    c0s = [0]
    for n in sizes:
        c0s.append(c0s[-1] + n)
    reg_off_rows = [0]
    for n in sizes:
        reg_off_rows.append(reg_off_rows[-1] + w * n * 128)
    r_split = reg_off_rows[1]
    assert len(sizes) == 2  # lo/hi gather tables == the two AG chunk tensors

    nc = bacc.Bacc(
        "TRN2", target_bir_lowering=False, debug=False, num_devices=w,
        num_swdge_queues=4,
    )

    # ---- external I/O -----------------------------------------------------
    xT_d = nc.dram_tensor("xT", [128, np_pad], BF16, kind="ExternalInput")
    W1_d = nc.dram_tensor("W1", [128, 128], F32, kind="ExternalInput")
    W2_d = nc.dram_tensor("W2", [128, 128], F32, kind="ExternalInput")
    w11T_d = nc.dram_tensor("w11T", [128, 128], F32, kind="ExternalInput")
    w12T_d = nc.dram_tensor("w12T", [128, 128], F32, kind="ExternalInput")
    w21T_d = nc.dram_tensor("w21T", [128, 128], F32, kind="ExternalInput")
    w22T_d = nc.dram_tensor("w22T", [128, 128], F32, kind="ExternalInput")
    b1c_d = nc.dram_tensor("b1c", [128, 1], F32, kind="ExternalInput")
    b2c_d = nc.dram_tensor("b2c", [128, 1], F32, kind="ExternalInput")
    bc1r_d = nc.dram_tensor("bc1r", [128, 128], F32, kind="ExternalInput")
    bc2r_d = nc.dram_tensor("bc2r", [128, 128], F32, kind="ExternalInput")
    dof_d = nc.dram_tensor("dof", [128, t_own], F32, kind="ExternalInput")
    dor_d = nc.dram_tensor("dor", [128, t_own], F32, kind="ExternalInput")
    dallf_d = nc.dram_tensor("dallf", [128, t_all], F32, kind="ExternalInput")
    dallr_d = nc.dram_tensor("dallr", [128, t_all], F32, kind="ExternalInput")
    ilof_d = nc.dram_tensor("ilof", [128, lo_cols], I16, kind="ExternalInput")
    ihif_d = nc.dram_tensor("ihif", [128, hi_cols], I16, kind="ExternalInput")
    ilor_d = nc.dram_tensor("ilor", [128, lo_cols], I16, kind="ExternalInput")
    ihir_d = nc.dram_tensor("ihir", [128, hi_cols], I16, kind="ExternalInput")
    dstf_d = nc.dram_tensor("dstf", [128, t_own * bt], BF16, kind="ExternalInput")
    dstr_d = nc.dram_tensor("dstr", [128, t_own * bt], BF16, kind="ExternalInput")
    out_d = nc.dram_tensor("out", [sh_pad, 128], F32, kind="ExternalOutput")

    from contextlib import ExitStack

    with tile.TileContext(nc) as tc, ExitStack() as ctx:
        sb = ctx.enter_context(tc.tile_pool(name="sb", bufs=1))
        ps = ctx.enter_context(tc.tile_pool(name="ps", bufs=1, space="PSUM"))
        dr = ctx.enter_context(tc.tile_pool(name="dr", bufs=1, space="DRAM"))

        # message tables: per (layer, direction) pre-scaled by dinv_dir[u],
        # computed REPLICATED on every core (no collective); lo/hi gather
        # sources are row-slices of one tensor.
        T1f = dr.tile([np_pad, 128], BF16, name="T1f")
        T1r = dr.tile([np_pad, 128], BF16, name="T1r")
        T2f = dr.tile([np_pad, 128], BF16, name="T2f")
        T2r = dr.tile([np_pad, 128], BF16, name="T2r")
        # gated layer-1 output hT: own chunk slabs -> AllGathered (only coll.)
        HTO = [
            dr.tile([128, n * 128], BF16, name=f"HTO{j}")
            for j, n in enumerate(sizes)
        ]
        HTF = [
            dr.tile([w, 128, n * 128], BF16, name=f"HTF{j}", addr_space="Shared")
            for j, n in enumerate(sizes)
        ]

        # ---- constants / small persistent SBUF ----
        def load_const(dram, shape, dtype=F32, name=None):
            t = sb.tile(shape, dtype, name=name or dram.name + "_sb")
            nc.sync.dma_start(out=t[:], in_=dram[:])
            return t

        def load_cast_bf16(dram, name):
            t32 = sb.tile([128, 128], F32, name=name + "_f32")
            nc.sync.dma_start(out=t32[:], in_=dram[:])
            t16 = sb.tile([128, 128], BF16, name=name)
            nc.vector.tensor_copy(out=t16[:], in_=t32[:])
            return t16

        W1b = load_cast_bf16(W1_d, "W1b")
        W2b = load_cast_bf16(W2_d, "W2b")
        w11Tb = load_cast_bf16(w11T_d, "w11Tb")
        w12Tb = load_cast_bf16(w12T_d, "w12Tb")
        w21Tb = load_cast_bf16(w21T_d, "w21Tb")
        w22Tb = load_cast_bf16(w22T_d, "w22Tb")
        b1c = load_const(b1c_d, [128, 1], name="b1c")
        b2c = load_const(b2c_d, [128, 1], name="b2c")
        bc1r = load_const(bc1r_d, [128, 128], name="bc1r") if add_bc1 else None
        bc2r = load_const(bc2r_d, [128, 128], name="bc2r") if add_bc2 else None
        dof = load_const(dof_d, [128, t_own], name="dof")
        dor = load_const(dor_d, [128, t_own], name="dor")
        dstf = load_const(dstf_d, [128, t_own * bt], BF16, "dstf_sb")
        dstr = load_const(dstr_d, [128, t_own * bt], BF16, "dstr_sb")
        dallf = load_const(dallf_d, [128, t_all], F32, "dallf_sb")
        dallr = load_const(dallr_d, [128, t_all], F32, "dallr_sb")

        iota_i = sb.tile([128, 128], I32, name="iota_i")
        nc.gpsimd.iota(iota_i[:], pattern=[[1, 128]], base=0, channel_multiplier=0)
        iota_bf = sb.tile([128, 128], BF16, name="iota_bf")
        nc.vector.tensor_copy(out=iota_bf[:], in_=iota_i[:])

        ident_bf = sb.tile([128, 128], BF16, name="ident_bf")
        make_identity(nc, ident_bf[:])
        ident_f32 = sb.tile([128, 128], F32, name="ident_f32")
        make_identity(nc, ident_f32[:])

        # own tile t -> column offset in xT (chunk-major slot space)
        def own_col(t):
            for j in range(len(sizes)):
                if t < c0s[j + 1]:
                    # column index c is baked per-core via per-core input? No:
                    # xT is the FULL table; own columns depend on core id.
                    raise AssertionError
            raise AssertionError

        # NOTE: xT is full; each core must read its own columns, which depend
        # on the core id. SPMD programs are identical across cores, so we pass
        # the own x slab as a separate per-core input instead.
        # (own_col unused; kept for clarity)

        # ---- dense pass: replicated over ALL tiles ------------------------
        def dense_tile(g, src_bf, Wb, Tf, Tr):
            """src_bf: [128 d, 128 n] lhsT tile for global tile g; writes
            dinv-scaled rows of the forward/reverse tables."""
            ph = ps.tile([128, 128], F32, tag="phd", bufs=2)
            nc.tensor.matmul(out=ph[:], lhsT=src_bf[:], rhs=Wb[:], start=True, stop=True)
            hf = sb.tile([128, 128], BF16, tag="hbdf", bufs=2)
            nc.vector.tensor_scalar(
                out=hf[:],
                in0=ph[:],
                scalar1=dallf[:, g : g + 1],
                scalar2=None,
                op0=mybir.AluOpType.mult,
            )
            nc.sync.dma_start(out=Tf[g * 128 : (g + 1) * 128, :], in_=hf[:])
            hr = sb.tile([128, 128], BF16, tag="hbdr", bufs=2)
            nc.scalar.activation(
                out=hr[:],
                in_=ph[:],
                func=mybir.ActivationFunctionType.Copy,
                scale=dallr[:, g : g + 1],
            )
            nc.sync.dma_start(out=Tr[g * 128 : (g + 1) * 128, :], in_=hr[:])

        def ag_chunk(j):
            nc.gpsimd.collective_compute(
                "AllGather",
                mybir.AluOpType.bypass,
                replica_groups=[list(range(w))],
                ins=[HTO[j].opt()],
                outs=[HTF[j].opt()],
            )

        # layer-1 dense: all tiles from the replicated x
        for g in range(t_all):
            xa = sb.tile([128, 128], BF16, tag="xa", bufs=3)
            nc.sync.dma_start(out=xa[:], in_=xT_d[:, g * 128 : (g + 1) * 128])
            dense_tile(g, xa, W1b, T1f, T1r)

        kdbg = os.environ.get("KDBG", "")
        if kdbg == "t1":
            dbg0 = nc.dram_tensor("dbg0", [r_split, 128], BF16, kind="ExternalOutput")
            dbg1 = nc.dram_tensor(
                "dbg1", [np_pad - r_split, 128], BF16, kind="ExternalOutput"
            )
            dbg2 = nc.dram_tensor("dbg2", [sh_pad, 128], BF16, kind="ExternalOutput")
            nc.sync.dma_start(out=dbg0[:], in_=T1f[0][:])
            nc.sync.dma_start(out=dbg1[:], in_=T1f[1][:])
            nc.sync.dma_start(out=dbg2[:], in_=OWN1f[:])

        # ---- gather machinery --------------------------------------------
        gq_sem = [nc.alloc_semaphore(f"gq{q}") for q in range(4)]

        idx_src = dict(f=(ilof_d, ihif_d), r=(ilor_d, ihir_d))

        use_prep = bool(int(os.environ.get("KPREP", "0")))

        def prep_batch(g, Tpair, tag):
            """Emit idx loads + gathers for batch g, both dirs.
            Returns dict dir -> msg tile."""
            kg = kg_of[g]
            msgs = {}
            for di, dname in enumerate("fr"):
                T = Tpair[di]
                lo_d, hi_d = idx_src[dname]
                msg = sb.tile([128, kg * bt, 128], BF16, tag=f"msg{dname}", bufs=3)
                ilo = sb.tile([128, kg * b_lo * 8], I16, tag=f"ilo{dname}", bufs=3)
                nc.sync.dma_start(out=ilo[:], in_=lo_d[:, lo_off[g] : lo_off[g + 1]])
                ihi = sb.tile([128, kg * b_hi * 8], I16, tag=f"ihi{dname}", bufs=3)
                nc.sync.dma_start(out=ihi[:], in_=hi_d[:, hi_off[g] : hi_off[g + 1]])
                q_lo = di * 2
                q_hi = di * 2 + 1
                pk = dict(prepare_only=True) if use_prep else {}
                nc.gpsimd.dma_gather(
                    out_ap=msg[:, : kg * b_lo, :],
                    in_ap=T[0:r_split, :],
                    idxs_ap=ilo[:],
                    num_idxs=kg * b_lo * 128,
                    num_idxs_reg=kg * b_lo * 128,
                    elem_size=128,
                    single_packet=False,
                    queue_num=q_lo,
                    sem=gq_sem[q_lo] if use_prep else None,
                    **pk,
                )
                nc.gpsimd.dma_gather(
                    out_ap=msg[:, kg * b_lo :, :],
                    in_ap=T[r_split:, :],
                    idxs_ap=ihi[:],
                    num_idxs=kg * b_hi * 128,
                    num_idxs_reg=kg * b_hi * 128,
                    elem_size=128,
                    single_packet=False,
                    queue_num=q_hi,
                    sem=gq_sem[q_hi] if use_prep else None,
                    **pk,
                )
                msgs[dname] = msg
            return msgs

        def fire_batch():
            if use_prep:
                for q in range(4):
                    nc.gpsimd.trigger_dma(count=None, queue_num=q)

        def msg_block(msg, kg, ti, b):
            if b < b_lo:
                return msg[:, ti * b_lo + b, :]
            return msg[:, kg * b_lo + ti * b_hi + (b - b_lo), :]

        def agg_tile(t, kg, ti, msg, dst_sb, dinvo, bcr, out_dtype, tagsuf):
            # one-hot S[e, b, j] = (iota j == dst_code[e, b])
            S = sb.tile([128, bt, 128], BF16, tag="S" + tagsuf, bufs=2)
            nc.vector.tensor_tensor(
                out=S[:],
                in0=iota_bf[:]
                .rearrange("p (o d) -> p o d", o=1)
                .to_broadcast([128, bt, 128]),
                in1=dst_sb[:, t * bt : (t + 1) * bt].to_broadcast([128, bt, 128]),
                op=mybir.AluOpType.is_equal,
            )
            agg = ps.tile([128, 128], F32, tag="agg", bufs=2)
            for b in range(bt):
                nc.tensor.matmul(
                    out=agg[:],
                    lhsT=S[:, b, :],
                    rhs=msg_block(msg, kg, ti, b),
                    start=(b == 0),
                    stop=(b == bt - 1),
                )
            if bcr is not None:
                s2 = sb.tile([128, 128], F32, tag="s2" + tagsuf, bufs=2)
                nc.vector.tensor_scalar(
                    out=s2[:],
                    in0=agg[:],
                    scalar1=dinvo[:, t : t + 1],
                    scalar2=None,
                    op0=mybir.AluOpType.mult,
                )
                s3 = sb.tile([128, 128], F32, tag="s3" + tagsuf, bufs=2)
                nc.vector.tensor_tensor(
                    out=s3[:], in0=s2[:], in1=bcr[:], op=mybir.AluOpType.add
                )
                od = sb.tile([128, 128], out_dtype, tag="od" + tagsuf, bufs=2)
                nc.scalar.activation(
                    out=od[:], in_=s3[:], func=mybir.ActivationFunctionType.Relu
                )
                return od
            od = sb.tile([128, 128], out_dtype, tag="od" + tagsuf, bufs=2)
            nc.scalar.activation(
                out=od[:],
                in_=agg[:],
                func=mybir.ActivationFunctionType.Relu,
                scale=dinvo[:, t : t + 1],
            )
            return od

        def transpose_to_bf16(src, ident, tagsuf):
            tp = ps.tile([128, 128], src.dtype, tag="tp", bufs=2)
            nc.tensor.transpose(out=tp[:], in_=src[:], identity=ident[:])
            oT = sb.tile([128, 128], BF16, tag="oT" + tagsuf, bufs=2)
            nc.vector.tensor_copy(out=oT[:], in_=tp[:])
            return oT

        def chunk_of(t):
            for j in range(len(sizes)):
                if t < c0s[j + 1]:
                    return j
            raise AssertionError

        # ---- layer 1 agg + gate + layer-2 dense ---------------------------
        dbg_o1 = dbg_o2 = dbg_ht = dbg_p1 = dbg_p2 = None
        if kdbg == "o1":
            dbg_o1 = nc.dram_tensor("dbgo1", [sh_pad, 128], F32, kind="ExternalOutput")
            dbg_o2 = nc.dram_tensor("dbgo2", [sh_pad, 128], F32, kind="ExternalOutput")
        if kdbg == "l2":
            dbg_ht = nc.dram_tensor("dbght", [sh_pad, 128], F32, kind="ExternalOutput")
            dbg_p1 = nc.dram_tensor("dbgp1", [sh_pad, 128], F32, kind="ExternalOutput")
            dbg_p2 = nc.dram_tensor("dbgp2", [sh_pad, 128], F32, kind="ExternalOutput")
        pending = prep_batch(0, (T1f, T1r), "1")
        ag2_done = [False] * len(sizes)
        for g in range(nb):
            fire_batch()
            # AllGather hT chunks whose tiles completed in prior batches.
            # Emitted here (no prep pending on any queue) so no collective
            # sits between a prepare_only and its trigger.
            for j in range(len(sizes)):
                if not ag2_done[j] and c0s[j + 1] <= g * k:
                    ag_chunk(j)
                    ag2_done[j] = True
            msgs = pending
            # prep next batch; layer-2 batch 0 must wait until T2's AllGather
            # chunks are EMITTED (Tile deps are emission-ordered), so it is
            # primed after this loop instead.
            if g + 1 < nb:
                pending = prep_batch(g + 1, (T1f, T1r), "1")
            kg = kg_of[g]
            for ti in range(kg):
                t = g * k + ti
                o1 = agg_tile(t, kg, ti, msgs["f"], dstf, dof, bc1r, BF16, "f")
                o2 = agg_tile(t, kg, ti, msgs["r"], dstr, dor, bc1r, BF16, "r")
                if dbg_o1 is not None:
                    o1f = sb.tile([128, 128], F32, tag="o1f", bufs=2)
                    nc.vector.tensor_copy(out=o1f[:], in_=o1[:])
                    nc.sync.dma_start(out=dbg_o1[t * 128 : (t + 1) * 128, :], in_=o1f[:])
                    o2f = sb.tile([128, 128], F32, tag="o2f", bufs=2)
                    nc.vector.tensor_copy(out=o2f[:], in_=o2[:])
                    nc.sync.dma_start(out=dbg_o2[t * 128 : (t + 1) * 128, :], in_=o2f[:])
                o1T = transpose_to_bf16(o1, ident_bf, "1")
                o2T = transpose_to_bf16(o2, ident_bf, "2")
                zps = ps.tile([128, 128], F32, tag="z", bufs=2)
                nc.tensor.matmul(out=zps[:], lhsT=w11Tb[:], rhs=o1T[:], start=True, stop=False)
                nc.tensor.matmul(out=zps[:], lhsT=w12Tb[:], rhs=o2T[:], start=False, stop=True)
                GT = sb.tile([128, 128], BF16, tag="GT", bufs=2)
                nc.scalar.activation(
                    out=GT[:],
                    in_=zps[:],
                    func=mybir.ActivationFunctionType.Sigmoid,
                    bias=b1c[:, :1],
                )
                dT = sb.tile([128, 128], BF16, tag="dT", bufs=2)
                nc.vector.tensor_tensor(
                    out=dT[:], in0=o1T[:], in1=o2T[:], op=mybir.AluOpType.subtract
                )
                pT = sb.tile([128, 128], BF16, tag="pT", bufs=2)
                nc.vector.tensor_tensor(
                    out=pT[:], in0=GT[:], in1=dT[:], op=mybir.AluOpType.mult
                )
                hT = sb.tile([128, 128], BF16, tag="hT", bufs=2)
                nc.vector.tensor_tensor(
                    out=hT[:], in0=pT[:], in1=o2T[:], op=mybir.AluOpType.add
                )
                if kdbg == "l2":
                    htf = sb.tile([128, 128], F32, tag="htf", bufs=2)
                    nc.vector.tensor_copy(out=htf[:], in_=hT[:])
                    nc.sync.dma_start(
                        out=dbg_ht[t * 128 : (t + 1) * 128, :], in_=htf[:]
                    )
                # stage own hT into the AllGather input slab for layer 2
                j = chunk_of(t)
                nc.sync.dma_start(
                    out=HTO[j][:, (t - c0s[j]) * 128 : (t - c0s[j] + 1) * 128],
                    in_=hT[:],
                )

        for j in range(len(sizes)):
            if not ag2_done[j]:
                ag_chunk(j)
                ag2_done[j] = True

        # layer-2 dense: all tiles from the AllGathered hT (lhsT directly)
        for j in range(len(sizes)):
            for r in range(w):
                for tl in range(c0s[j], c0s[j + 1]):
                    # global tile id in slot space: region j, core r, tile tl
                    gt = reg_off_rows[j] // 128 + r * sizes[j] + (tl - c0s[j])
                    ha = sb.tile([128, 128], BF16, tag="xa", bufs=3)
                    nc.sync.dma_start(
                        out=ha[:],
                        in_=HTF[j][r, :, (tl - c0s[j]) * 128 : (tl - c0s[j] + 1) * 128],
                    )
                    dense_tile(gt, ha, W2b, T2f, T2r)

        if kdbg == "l2":
            dbg_t20 = nc.dram_tensor("dbgt20", [r_split, 128], BF16, kind="ExternalOutput")
            dbg_t21 = nc.dram_tensor(
                "dbgt21", [np_pad - r_split, 128], BF16, kind="ExternalOutput"
            )
            nc.sync.dma_start(out=dbg_t20[:], in_=T2f[0][:])
            nc.sync.dma_start(out=dbg_t21[:], in_=T2f[1][:])

        # ---- layer 2 agg + gate + output ---------------------------------
        pending = prep_batch(0, (T2f, T2r), "2")
        for g in range(nb):
            fire_batch()
            msgs = pending
            if g + 1 < nb:
                pending = prep_batch(g + 1, (T2f, T2r), "2")
            kg = kg_of[g]
            for ti in range(kg):
                t = g * k + ti
                p1 = agg_tile(t, kg, ti, msgs["f"], dstf, dof, bc2r, F32, "f")
                p2 = agg_tile(t, kg, ti, msgs["r"], dstr, dor, bc2r, F32, "r")
                if dbg_p1 is not None:
                    nc.sync.dma_start(out=dbg_p1[t * 128 : (t + 1) * 128, :], in_=p1[:])
                    nc.sync.dma_start(out=dbg_p2[t * 128 : (t + 1) * 128, :], in_=p2[:])
                p1T = transpose_to_bf16(p1, ident_f32, "1")
                p2T = transpose_to_bf16(p2, ident_f32, "2")
                zps = ps.tile([128, 128], F32, tag="z", bufs=2)
                nc.tensor.matmul(out=zps[:], lhsT=w21Tb[:], rhs=p1T[:], start=True, stop=False)
                nc.tensor.matmul(out=zps[:], lhsT=w22Tb[:], rhs=p2T[:], start=False, stop=True)
                G2T = sb.tile([128, 128], BF16, tag="GT", bufs=2)
                nc.scalar.activation(
                    out=G2T[:],
                    in_=zps[:],
                    func=mybir.ActivationFunctionType.Sigmoid,
                    bias=b2c[:, :1],
                )
                g2p = ps.tile([128, 128], BF16, tag="tp", bufs=2)
                nc.tensor.transpose(out=g2p[:], in_=G2T[:], identity=ident_bf[:])
                g2s = sb.tile([128, 128], F32, tag="g2s", bufs=2)
                nc.vector.tensor_copy(out=g2s[:], in_=g2p[:])
                dd = sb.tile([128, 128], F32, tag="dd", bufs=2)
                nc.vector.tensor_tensor(
                    out=dd[:], in0=p1[:], in1=p2[:], op=mybir.AluOpType.subtract
                )
                pr = sb.tile([128, 128], F32, tag="pr", bufs=2)
                nc.vector.tensor_tensor(
                    out=pr[:], in0=dd[:], in1=g2s[:], op=mybir.AluOpType.mult
                )
                ot = sb.tile([128, 128], F32, tag="ot", bufs=2)
                nc.vector.tensor_tensor(
                    out=ot[:], in0=pr[:], in1=p2[:], op=mybir.AluOpType.add
                )
                nc.sync.dma_start(out=out_d[t * 128 : (t + 1) * 128, :], in_=ot[:])

    nc.compile()
    return nc


# ---------------------------------------------------------------------------
# full pipeline
# ---------------------------------------------------------------------------


def make_in_maps(inputs, meta, tables):
    import ml_dtypes

    w = W_CORES
    sh_real, sh_pad = meta["sh_real"], meta["sh_pad"]
    t_own, np_pad = meta["t_own"], meta["np_pad"]
    n_real = w * sh_real
    slot_of = meta["slot_of"]

    x = np.asarray(inputs["x"], np.float32)
    slots = slot_of(np.arange(n_real))
    x_slot = np.zeros((np_pad, D), np.float32)
    x_slot[slots] = x
    xT = np.ascontiguousarray(x_slot.T).astype(ml_dtypes.bfloat16)

    def t2(a):
        return np.ascontiguousarray(np.asarray(a, np.float32).T)

    W1 = np.asarray(inputs["W1"], np.float32)
    W2 = np.asarray(inputs["W2"], np.float32)
    b1c = np.asarray(inputs["b1"], np.float32).reshape(128, 1)
    b2c = np.asarray(inputs["b2"], np.float32).reshape(128, 1)
    bc1r = np.broadcast_to(np.asarray(inputs["bc1"], np.float32), (128, 128)).copy()
    bc2r = np.broadcast_to(np.asarray(inputs["bc2"], np.float32), (128, 128)).copy()

    in_maps = []
    for c in range(w):
        in_maps.append(
            dict(
                xT=xT,
                W1=W1, W2=W2,
                w11T=t2(inputs["w11"]), w12T=t2(inputs["w12"]),
                w21T=t2(inputs["w21"]), w22T=t2(inputs["w22"]),
                b1c=b1c, b2c=b2c, bc1r=bc1r, bc2r=bc2r,
                dof=tables["dof"][c], dor=tables["dor"][c],
                dallf=tables["dallf"], dallr=tables["dallr"],
                ilof=tables["ilo_f"][c], ihif=tables["ihi_f"][c],
                ilor=tables["ilo_r"][c], ihir=tables["ihi_r"][c],
                dstf=tables["dst_f"][c], dstr=tables["dst_r"][c],
            )
        )
    return in_maps


def assemble_output(results, meta):
    sh_real, sh_pad = meta["sh_real"], meta["sh_pad"]
    n_real = W_CORES * sh_real
    full = np.concatenate([r["out"] for r in results], axis=0)
    v = np.arange(n_real)
    rows = (v // sh_real) * sh_pad + (v % sh_real)
    return np.ascontiguousarray(full[rows]).astype(np.float32)


_CACHE = {}


def _get_program(meta, add_bc1, add_bc2):
    key = (meta["t_own"], meta["b_lo"], meta["b_hi"], meta["k"], add_bc1, add_bc2)
    if key not in _CACHE:
        _CACHE[key] = build_program(
            meta["t_own"], meta["b_lo"], meta["b_hi"], meta["k"],
            add_bc1=add_bc1, add_bc2=add_bc2,
        )
    return _CACHE[key]


def _install_ntff_hook():
    """Shim antenv.axon_hooks (absent in this image) so run_bass_kernel_spmd
    trace=True can capture NTFF profiles via libaxon_pjrt.so ctypes calls."""
    import contextlib
    import ctypes
    import types

    if "antenv.axon_hooks" in sys.modules:
        return
    so_path = "/opt/axon/libaxon_pjrt.so"
    holder = {}
    m = types.ModuleType("antenv.axon_hooks")
    m.set_axon_ntff_profile_hook = lambda h: holder.__setitem__("h", h)
    m.get_axon_ntff_profile_hook = lambda: holder.get("h")
    sys.modules["antenv.axon_hooks"] = m
    try:
        import antenv

        antenv.axon_hooks = m
    except ImportError:
        pass
    try:
        lib = ctypes.CDLL(so_path)
    except OSError:
        return
    if not hasattr(lib, "axon_start_nrt_profile"):
        return
    lib.axon_start_nrt_profile.argtypes = [
        ctypes.POINTER(ctypes.c_int64),
        ctypes.c_size_t,
    ]
    lib.axon_start_nrt_profile.restype = ctypes.c_int64
    lib.axon_stop_nrt_profile.argtypes = [ctypes.c_char_p]
    lib.axon_stop_nrt_profile.restype = ctypes.c_int64

    @contextlib.contextmanager
    def _hook(output_dir, device_ids):
        import jax

        jax.devices()
        if device_ids:
            ids = (ctypes.c_int64 * len(device_ids))(*device_ids)
            rc = lib.axon_start_nrt_profile(ids, len(device_ids))
        else:
            rc = lib.axon_start_nrt_profile(None, 0)
        if rc != 0:
            raise RuntimeError(f"axon_start_nrt_profile rc={rc}")
        try:
            yield
        finally:
            n = lib.axon_stop_nrt_profile(str(output_dir).encode())
            print(f"profile: {n} file(s) written to {output_dir}", file=sys.stderr)

    holder["h"] = _hook


def _patch_upload_artifacts():
    import concourse.bass_utils as bu

    bu.upload_artifacts = lambda tmpdir: tmpdir


def kernel(**inputs):
    x = np.asarray(inputs["x"], np.float32)
    n_real = x.shape[0]
    meta, tables = host_prepare(x, np.asarray(inputs["edge_index"]), n_real)
    add_bc1 = bool(np.any(np.asarray(inputs["bc1"]) != 0))
    add_bc2 = bool(np.any(np.asarray(inputs["bc2"]) != 0))
    nc = _get_program(meta, add_bc1, add_bc2)
    in_maps = make_in_maps(inputs, meta, tables)
    if bool(int(os.environ.get("KERNEL_TRACE", "0"))):
        _install_ntff_hook()
        _patch_upload_artifacts()
    res = run_bass_kernel_spmd(
        nc,
        in_maps,
        core_ids=list(range(W_CORES)),
        trace=bool(int(os.environ.get("KERNEL_TRACE", "0"))),
    )
    global LAST_EXEC_NS
    LAST_EXEC_NS = res.exec_time_ns
    if res.exec_time_ns is not None:
        print(f"HW exec time: {res.exec_time_ns} ns")
    return assemble_output(res.results, meta)


LAST_EXEC_NS = None


# revision 30
# speedup vs baseline: 1.2759x; 1.2759x over previous
"""Trainium2 Bass kernel for a 2-layer dual-direction gated GCN (DGGCN).

Contract: kernel(**inputs) takes the FULL unsharded inputs (as produced by
setup_inputs) and returns the FULL [N, D] float32 output.

Strategy (8 NeuronCores, node partition):
  - Nodes are remapped into a chunk-major padded slot space: the node axis is
    split into ag_chunks regions; within a region, cores' tiles are laid out
    contiguously per core so a chunked AllGather writes each region with one
    contiguous collective. Self-loops are explicit (v, v) edges. Edges are
    bucketed by destination tile (forward) / source tile (reverse) and split
    per tile into lo/hi halves of the slot space (region boundary) so gather
    indices fit dma_gather's int16 format. Uniform B_LO/B_HI block counts per
    tile keep the program SPMD.
  - Per layer each core computes h @ W for OWN tiles only, writes its slab,
    and a chunked AllGather replicates the bf16 message table to all cores.
    dinv[src] is NOT folded into the table; instead it is folded into the
    one-hot scatter matrices, so one table serves both edge directions.
  - Aggregation per own dst tile: batched dma_gather of table rows. All
    gathers are issued as prepare_only descriptor-generation (GPSIMD) one
    batch ahead and fired with trigger_dma, so SWDGE generation overlaps
    compute/DMA. Scatter matrices are built on-chip with a fused
    tensor_scalar (iota == dst_code) * dinv_src, then PE matmuls accumulate
    segment sums in PSUM; relu(x)*dinv_dst is fused on ACT.
"""

import os
import sys

sys.path.insert(0, "/opt/trn_rl_repo")

import numpy as np

import concourse.bacc as bacc
import concourse.bass as bass
import concourse.tile as tile
from concourse import mybir
from concourse.bass_utils import run_bass_kernel_spmd
from concourse.masks import make_identity

F32 = mybir.dt.float32
BF16 = mybir.dt.bfloat16
I32 = mybir.dt.int32
I16 = mybir.dt.int16

W_CORES = 8
D = 128
PAD_DST = 200.0  # sentinel local-dst id (never matches iota 0..127)
AG_CHUNKS = 2


# ---------------------------------------------------------------------------
# host-side graph preprocessing (index bucketing / sharding metadata only)
# ---------------------------------------------------------------------------


def _chunk_sizes(t_own):
    sizes = []
    base = 0
    for j in range(AG_CHUNKS):
        n = (t_own - base) // (AG_CHUNKS - j)
        if n > 0:
            sizes.append(n)
            base += n
    return sizes


def _pack16(flat):
    """Pack an int array [n] into dma_gather's [128, n//16] int16 layout:
    index i lives at partition i%16, column i//16, replicated to 8 stripes."""
    n = flat.shape[0]
    assert n % 16 == 0
    return np.tile(flat.reshape(n // 16, 16).T, (8, 1))


def host_prepare(x, edge_index, n_real):
    w = W_CORES
    assert n_real % w == 0
    sh_real = n_real // w
    t_own = (sh_real + 127) // 128
    sh_pad = t_own * 128
    t_all = w * t_own
    np_pad = t_all * 128
    sizes = _chunk_sizes(t_own)
    # chunk-major region starts (rows) and per-core tile starts within chunks
    reg_rows = [w * n * 128 for n in sizes]
    reg_off = np.concatenate([[0], np.cumsum(reg_rows)]).astype(np.int64)
    r_split = int(reg_off[1])  # lo/hi gather split at region-0 boundary
    assert r_split <= 32767 + 1 and (np_pad - r_split) <= 32767 + 1
    chunk_of_tile = np.concatenate(
        [np.full(n, j, np.int64) for j, n in enumerate(sizes)]
    )
    tile_in_chunk = np.concatenate([np.arange(n) for n in sizes])
    c0_of_chunk = np.concatenate([[0], np.cumsum(sizes)]).astype(np.int64)

    sizes_arr = np.asarray(sizes, np.int64)

    def slot_of(v):
        c = v // sh_real
        l = v % sh_real
        t = l // 128
        off = l % 128
        j = chunk_of_tile[t]
        return reg_off[j] + (c * sizes_arr[j] + tile_in_chunk[t]) * 128 + off

    src = np.asarray(edge_index[0], dtype=np.int64)
    dst = np.asarray(edge_index[1], dtype=np.int64)
    ss = slot_of(src)
    ds = slot_of(dst)
    selfs = slot_of(np.arange(n_real, dtype=np.int64))

    # degrees (+1 self-loop) in slot space; dinv on host
    deg_f = np.ones(np_pad, np.float32)
    deg_r = np.ones(np_pad, np.float32)
    np.add.at(deg_f, ds, 1.0)
    np.add.at(deg_r, ss, 1.0)
    dinv_f = (1.0 / np.sqrt(deg_f)).astype(np.float32)
    dinv_r = (1.0 / np.sqrt(deg_r)).astype(np.float32)

    # global tile id of a slot (for bucketing): region-aware
    def tile_of_slot(s):
        out = np.empty(s.shape, np.int64)
        for j in range(len(sizes)):
            m = (s >= reg_off[j]) & (s < reg_off[j + 1])
            rel = (s[m] - reg_off[j]) // 128  # core-major within region
            c = rel // sizes[j]
            t = rel % sizes[j]
            out[m] = c * t_own + c0_of_chunk[j] + t
        return out

    # self-loops as explicit edges
    agg_f = np.concatenate([ds, selfs])
    gat_f = np.concatenate([ss, selfs])
    agg_r = np.concatenate([ss, selfs])
    gat_r = np.concatenate([ds, selfs])

    def bucket(agg_slot, gather_slot):
        tile_id = tile_of_slot(agg_slot)
        hi = (gather_slot >= r_split).astype(np.int64)
        order = np.lexsort((hi, tile_id))
        t_s = tile_id[order]
        g_s = gather_slot[order]
        h_s = hi[order]
        loc_s = (agg_slot[order] % 128).astype(np.float32)
        n_lo = np.bincount(t_s[h_s == 0], minlength=t_all)
        n_hi = np.bincount(t_s[h_s == 1], minlength=t_all)
        return t_s, g_s, h_s, loc_s, n_lo, n_hi

    bf = bucket(agg_f, gat_f)
    br = bucket(agg_r, gat_r)
    b_lo = int(max(bf[4].max(), br[4].max()) + 127) // 128
    b_hi = int(max(bf[5].max(), br[5].max()) + 127) // 128
    bt = b_lo + b_hi

    def build_tables(t_s, g_s, h_s, loc_s, n_lo, n_hi):
        idx_lo = np.zeros((t_all, b_lo * 128), np.int16)
        idx_hi = np.zeros((t_all, b_hi * 128), np.int16)
        dst_t = np.full((t_all, bt * 128), PAD_DST, np.float32)
        n_edges = len(t_s)
        tile_starts = np.zeros(t_all + 1, np.int64)
        np.cumsum(n_lo + n_hi, out=tile_starts[1:])
        pos_in_tile = np.arange(n_edges) - tile_starts[t_s]
        pos_lo = pos_in_tile
        pos_hi = pos_in_tile - n_lo[t_s]
        mlo = h_s == 0
        mhi = h_s == 1
        idx_lo[t_s[mlo], pos_lo[mlo]] = g_s[mlo].astype(np.int16)
        idx_hi[t_s[mhi], pos_hi[mhi]] = (g_s[mhi] - r_split).astype(np.int16)
        dst_t[t_s[mlo], pos_lo[mlo]] = loc_s[mlo]
        dst_t[t_s[mhi], b_lo * 128 + pos_hi[mhi]] = loc_s[mhi]
        return idx_lo, idx_hi, dst_t

    tbl_f = build_tables(*bf)
    tbl_r = build_tables(*br)

    k = min(3, t_own)
    nb = (t_own + k - 1) // k

    def per_core(idx_lo, idx_hi, dst_t):
        import ml_dtypes
        idxlo_l, idxhi_l, dst_l = [], [], []
        for c in range(w):
            sl = slice(c * t_own, (c + 1) * t_own)
            ilo = idx_lo[sl]
            ihi = idx_hi[sl]
            plo = np.concatenate(
                [_pack16(ilo[g * k : min((g + 1) * k, t_own)].reshape(-1)) for g in range(nb)],
                axis=1,
            )
            phi = np.concatenate(
                [_pack16(ihi[g * k : min((g + 1) * k, t_own)].reshape(-1)) for g in range(nb)],
                axis=1,
            )

            def t_pack(a):
                return np.ascontiguousarray(
                    a.reshape(t_own, bt, 128).transpose(2, 0, 1).reshape(128, t_own * bt)
                ).astype(ml_dtypes.bfloat16)

            idxlo_l.append(np.ascontiguousarray(plo))
            idxhi_l.append(np.ascontiguousarray(phi))
            dst_l.append(t_pack(dst_t[sl]))
        return idxlo_l, idxhi_l, dst_l

    ilo_f, ihi_f, dst_f = per_core(*tbl_f)
    ilo_r, ihi_r, dst_r = per_core(*tbl_r)

    # own-tile dinv columns [128, t_own] per core per direction
    def own_dinv(dinv):
        cols = []
        for c in range(w):
            rows = np.empty(sh_pad, np.float32)
            for t in range(t_own):
                j = chunk_of_tile[t]
                r0 = reg_off[j] + (c * sizes[j] + tile_in_chunk[t]) * 128
                rows[t * 128 : (t + 1) * 128] = dinv[r0 : r0 + 128]
            cols.append(np.ascontiguousarray(rows.reshape(t_own, 128).T))
        return cols

    dof = own_dinv(dinv_f)
    dor = own_dinv(dinv_r)

    meta = dict(
        sh_real=sh_real, sh_pad=sh_pad, t_own=t_own, t_all=t_all, np_pad=np_pad,
        b_lo=b_lo, b_hi=b_hi, k=k, nb=nb, sizes=sizes, r_split=r_split,
        slot_of=slot_of,
    )
    tables = dict(
        ilo_f=ilo_f, ihi_f=ihi_f, dst_f=dst_f,
        ilo_r=ilo_r, ihi_r=ihi_r, dst_r=dst_r,
        dof=dof, dor=dor,
        dallf=np.ascontiguousarray(dinv_f.reshape(t_all, 128).T),
        dallr=np.ascontiguousarray(dinv_r.reshape(t_all, 128).T),
    )
    return meta, tables


# ---------------------------------------------------------------------------
# device program
# ---------------------------------------------------------------------------


def build_program(t_own, b_lo, b_hi, k, add_bc1=False, add_bc2=False):
    w = W_CORES
    t_all = w * t_own
    np_pad = t_all * 128
    sh_pad = t_own * 128
    bt = b_lo + b_hi
    nb = (t_own + k - 1) // k
    kg_of = [min(k, t_own - g * k) for g in range(nb)]
    lo_off = [sum(kg_of[:g]) * b_lo * 8 for g in range(nb + 1)]
    hi_off = [sum(kg_of[:g]) * b_hi * 8 for g in range(nb + 1)]
    lo_cols = lo_off[-1]
    hi_cols = hi_off[-1]
    sizes = _chunk_sizes(t_own)
    c0s = [0]
    for n in sizes:
        c0s.append(c0s[-1] + n)
    reg_off_rows = [0]
    for n in sizes:
        reg_off_rows.append(reg_off_rows[-1] + w * n * 128)
    r_split = reg_off_rows[1]
    assert len(sizes) == 2  # lo/hi gather tables == the two AG chunk tensors

    nc = bacc.Bacc(
        "TRN2", target_bir_lowering=False, debug=False, num_devices=w,
        num_swdge_queues=4,
    )

    # ---- external I/O -----------------------------------------------------
    xT_d = nc.dram_tensor("xT", [128, np_pad], BF16, kind="ExternalInput")
    W1_d = nc.dram_tensor("W1", [128, 128], F32, kind="ExternalInput")
    W2_d = nc.dram_tensor("W2", [128, 128], F32, kind="ExternalInput")
    w11T_d = nc.dram_tensor("w11T", [128, 128], F32, kind="ExternalInput")
    w12T_d = nc.dram_tensor("w12T", [128, 128], F32, kind="ExternalInput")
    w21T_d = nc.dram_tensor("w21T", [128, 128], F32, kind="ExternalInput")
    w22T_d = nc.dram_tensor("w22T", [128, 128], F32, kind="ExternalInput")
    b1c_d = nc.dram_tensor("b1c", [128, 1], F32, kind="ExternalInput")
    b2c_d = nc.dram_tensor("b2c", [128, 1], F32, kind="ExternalInput")
    bc1r_d = nc.dram_tensor("bc1r", [128, 128], F32, kind="ExternalInput")
    bc2r_d = nc.dram_tensor("bc2r", [128, 128], F32, kind="ExternalInput")
    dof_d = nc.dram_tensor("dof", [128, t_own], F32, kind="ExternalInput")
    dor_d = nc.dram_tensor("dor", [128, t_own], F32, kind="ExternalInput")
    dallf_d = nc.dram_tensor("dallf", [128, t_all], F32, kind="ExternalInput")
    dallr_d = nc.dram_tensor("dallr", [128, t_all], F32, kind="ExternalInput")
    ilof_d = nc.dram_tensor("ilof", [128, lo_cols], I16, kind="ExternalInput")
    ihif_d = nc.dram_tensor("ihif", [128, hi_cols], I16, kind="ExternalInput")
    ilor_d = nc.dram_tensor("ilor", [128, lo_cols], I16, kind="ExternalInput")
    ihir_d = nc.dram_tensor("ihir", [128, hi_cols], I16, kind="ExternalInput")
    dstf_d = nc.dram_tensor("dstf", [128, t_own * bt], BF16, kind="ExternalInput")
    dstr_d = nc.dram_tensor("dstr", [128, t_own * bt], BF16, kind="ExternalInput")
    out_d = nc.dram_tensor("out", [sh_pad, 128], F32, kind="ExternalOutput")

    from contextlib import ExitStack

    with tile.TileContext(nc) as tc, ExitStack() as ctx:
        sb = ctx.enter_context(tc.tile_pool(name="sb", bufs=1))
        ps = ctx.enter_context(tc.tile_pool(name="ps", bufs=1, space="PSUM"))
        dr = ctx.enter_context(tc.tile_pool(name="dr", bufs=1, space="DRAM"))

        # message tables: per (layer, direction) pre-scaled by dinv_dir[u],
        # computed REPLICATED on every core (no collective). Separate lo/hi
        # tensors so lo gathers only depend on the first half of the dense
        # pass (subrange deps are tensor-granular).
        def table(name):
            return (
                dr.tile([r_split, 128], BF16, name=name + "lo"),
                dr.tile([np_pad - r_split, 128], BF16, name=name + "hi"),
            )

        T1f, T1r = table("T1f"), table("T1r")
        T2f, T2r = table("T2f"), table("T2r")
        # gated layer-1 output hT: own chunk slabs -> AllGathered (only coll.)
        HTO = [
            dr.tile([128, n * 128], BF16, name=f"HTO{j}")
            for j, n in enumerate(sizes)
        ]
        HTF = [
            dr.tile([w, 128, n * 128], BF16, name=f"HTF{j}", addr_space="Shared")
            for j, n in enumerate(sizes)
        ]

        # ---- constants / small persistent SBUF ----
        def load_const(dram, shape, dtype=F32, name=None):
            t = sb.tile(shape, dtype, name=name or dram.name + "_sb")
            nc.sync.dma_start(out=t[:], in_=dram[:])
            return t

        def load_cast_bf16(dram, name):
            t32 = sb.tile([128, 128], F32, name=name + "_f32")
            nc.sync.dma_start(out=t32[:], in_=dram[:])
            t16 = sb.tile([128, 128], BF16, name=name)
            nc.vector.tensor_copy(out=t16[:], in_=t32[:])
            return t16

        W1b = load_cast_bf16(W1_d, "W1b")
        W2b = load_cast_bf16(W2_d, "W2b")
        w11Tb = load_cast_bf16(w11T_d, "w11Tb")
        w12Tb = load_cast_bf16(w12T_d, "w12Tb")
        w21Tb = load_cast_bf16(w21T_d, "w21Tb")
        w22Tb = load_cast_bf16(w22T_d, "w22Tb")
        b1c = load_const(b1c_d, [128, 1], name="b1c")
        b2c = load_const(b2c_d, [128, 1], name="b2c")
        bc1r = load_const(bc1r_d, [128, 128], name="bc1r") if add_bc1 else None
        bc2r = load_const(bc2r_d, [128, 128], name="bc2r") if add_bc2 else None
        dof = load_const(dof_d, [128, t_own], name="dof")
        dor = load_const(dor_d, [128, t_own], name="dor")
        dstf = load_const(dstf_d, [128, t_own * bt], BF16, "dstf_sb")
        dstr = load_const(dstr_d, [128, t_own * bt], BF16, "dstr_sb")
        dallf = load_const(dallf_d, [128, t_all], F32, "dallf_sb")
        dallr = load_const(dallr_d, [128, t_all], F32, "dallr_sb")

        iota_i = sb.tile([128, 128], I32, name="iota_i")
        nc.gpsimd.iota(iota_i[:], pattern=[[1, 128]], base=0, channel_multiplier=0)
        iota_bf = sb.tile([128, 128], BF16, name="iota_bf")
        nc.vector.tensor_copy(out=iota_bf[:], in_=iota_i[:])

        ident_bf = sb.tile([128, 128], BF16, name="ident_bf")
        make_identity(nc, ident_bf[:])
        ident_f32 = sb.tile([128, 128], F32, name="ident_f32")
        make_identity(nc, ident_f32[:])

        # own tile t -> column offset in xT (chunk-major slot space)
        def own_col(t):
            for j in range(len(sizes)):
                if t < c0s[j + 1]:
                    # column index c is baked per-core via per-core input? No:
                    # xT is the FULL table; own columns depend on core id.
                    raise AssertionError
            raise AssertionError

        # NOTE: xT is full; each core must read its own columns, which depend
        # on the core id. SPMD programs are identical across cores, so we pass
        # the own x slab as a separate per-core input instead.
        # (own_col unused; kept for clarity)

        # ---- dense pass: replicated over ALL tiles, slab-based ------------
        half_t = r_split // 128

        def dense_slab(g0, nt, xs_bf, Wb, Tf, Tr):
            """xs_bf: [128 d, nt*128 n] lhsT slab for tiles g0..g0+nt; writes
            dinv-scaled rows of the forward/reverse tables (one DMA each)."""
            hfs = sb.tile([128, nt * 128], BF16, tag="hfs", bufs=2)
            hrs = sb.tile([128, nt * 128], BF16, tag="hrs", bufs=2)
            for i in range(nt):
                g = g0 + i
                ph = ps.tile([128, 128], F32, tag="phd", bufs=2)
                nc.tensor.matmul(
                    out=ph[:],
                    lhsT=xs_bf[:, i * 128 : (i + 1) * 128],
                    rhs=Wb[:],
                    start=True,
                    stop=True,
                )
                nc.vector.tensor_scalar(
                    out=hfs[:, i * 128 : (i + 1) * 128],
                    in0=ph[:],
                    scalar1=dallf[:, g : g + 1],
                    scalar2=None,
                    op0=mybir.AluOpType.mult,
                )
                nc.scalar.activation(
                    out=hrs[:, i * 128 : (i + 1) * 128],
                    in_=ph[:],
                    func=mybir.ActivationFunctionType.Copy,
                    scale=dallr[:, g : g + 1],
                )
            hx = 0 if g0 < half_t else 1
            r0 = (g0 - hx * half_t) * 128
            for tbl, slb in ((Tf[hx], hfs), (Tr[hx], hrs)):
                nc.sync.dma_start(
                    out=tbl[r0 : r0 + nt * 128, :].rearrange("(t p) d -> p t d", p=128),
                    in_=slb[:].rearrange("p (t d) -> p t d", t=nt),
                )

        def ag_chunk(j):
            nc.gpsimd.collective_compute(
                "AllGather",
                mybir.AluOpType.bypass,
                replica_groups=[list(range(w))],
                ins=[HTO[j].opt()],
                outs=[HTF[j].opt()],
            )

        # layer-1 dense: all tiles from the replicated x (slabs never cross
        # the lo/hi boundary: half_t % slab == 0 enforced by slab choice)
        slab = 1
        for cand in range(min(8, half_t), 0, -1):
            if half_t % cand == 0 and (t_all - half_t) % cand == 0:
                slab = cand
                break
        for g0 in range(0, t_all, slab):
            xs = sb.tile([128, slab * 128], BF16, tag="xa", bufs=3)
            nc.sync.dma_start(
                out=xs[:], in_=xT_d[:, g0 * 128 : (g0 + slab) * 128]
            )
            dense_slab(g0, slab, xs, W1b, T1f, T1r)

        kdbg = os.environ.get("KDBG", "")
        if kdbg == "t1":
            dbg0 = nc.dram_tensor("dbg0", [r_split, 128], BF16, kind="ExternalOutput")
            dbg1 = nc.dram_tensor(
                "dbg1", [np_pad - r_split, 128], BF16, kind="ExternalOutput"
            )
            nc.sync.dma_start(out=dbg0[:], in_=T1f[0][:])
            nc.sync.dma_start(out=dbg1[:], in_=T1f[1][:])

        # ---- gather machinery --------------------------------------------
        gq_sem = [nc.alloc_semaphore(f"gq{q}") for q in range(4)]

        idx_src = dict(f=(ilof_d, ihif_d), r=(ilor_d, ihir_d))

        use_prep = bool(int(os.environ.get("KPREP", "0")))

        def prep_batch(g, Tpair, tag):
            """Emit idx loads + gathers for batch g, both dirs.
            Returns dict dir -> msg tile."""
            kg = kg_of[g]
            msgs = {}
            for di, dname in enumerate("fr"):
                T = Tpair[di]
                lo_d, hi_d = idx_src[dname]
                msg = sb.tile([128, kg * bt, 128], BF16, tag=f"msg{dname}", bufs=3)
                ilo = sb.tile([128, kg * b_lo * 8], I16, tag=f"ilo{dname}", bufs=3)
                nc.sync.dma_start(out=ilo[:], in_=lo_d[:, lo_off[g] : lo_off[g + 1]])
                ihi = sb.tile([128, kg * b_hi * 8], I16, tag=f"ihi{dname}", bufs=3)
                nc.sync.dma_start(out=ihi[:], in_=hi_d[:, hi_off[g] : hi_off[g + 1]])
                q_lo = di * 2
                q_hi = di * 2 + 1
                pk = dict(prepare_only=True) if use_prep else {}
                nc.gpsimd.dma_gather(
                    out_ap=msg[:, : kg * b_lo, :],
                    in_ap=T[0][:],
                    idxs_ap=ilo[:],
                    num_idxs=kg * b_lo * 128,
                    num_idxs_reg=kg * b_lo * 128,
                    elem_size=128,
                    single_packet=False,
                    queue_num=q_lo,
                    sem=gq_sem[q_lo] if use_prep else None,
                    **pk,
                )
                nc.gpsimd.dma_gather(
                    out_ap=msg[:, kg * b_lo :, :],
                    in_ap=T[1][:],
                    idxs_ap=ihi[:],
                    num_idxs=kg * b_hi * 128,
                    num_idxs_reg=kg * b_hi * 128,
                    elem_size=128,
                    single_packet=False,
                    queue_num=q_hi,
                    sem=gq_sem[q_hi] if use_prep else None,
                    **pk,
                )
                msgs[dname] = msg
            return msgs

        def fire_batch():
            if use_prep:
                for q in range(4):
                    nc.gpsimd.trigger_dma(count=None, queue_num=q)

        def msg_block(msg, kg, ti, b):
            if b < b_lo:
                return msg[:, ti * b_lo + b, :]
            return msg[:, kg * b_lo + ti * b_hi + (b - b_lo), :]

        def agg_tile(t, kg, ti, msg, dst_sb, dinvo, bcr, out_dtype, tagsuf):
            # one-hot S[e, b, j] = (iota j == dst_code[e, b])
            S = sb.tile([128, bt, 128], BF16, tag="S" + tagsuf, bufs=2)
            nc.vector.tensor_tensor(
                out=S[:],
                in0=iota_bf[:]
                .rearrange("p (o d) -> p o d", o=1)
                .to_broadcast([128, bt, 128]),
                in1=dst_sb[:, t * bt : (t + 1) * bt].to_broadcast([128, bt, 128]),
                op=mybir.AluOpType.is_equal,
            )
            agg = ps.tile([128, 128], F32, tag="agg", bufs=2)
            for b in range(bt):
                nc.tensor.matmul(
                    out=agg[:],
                    lhsT=S[:, b, :],
                    rhs=msg_block(msg, kg, ti, b),
                    start=(b == 0),
                    stop=(b == bt - 1),
                )
            if bcr is not None:
                s2 = sb.tile([128, 128], F32, tag="s2" + tagsuf, bufs=2)
                nc.vector.tensor_scalar(
                    out=s2[:],
                    in0=agg[:],
                    scalar1=dinvo[:, t : t + 1],
                    scalar2=None,
                    op0=mybir.AluOpType.mult,
                )
                s3 = sb.tile([128, 128], F32, tag="s3" + tagsuf, bufs=2)
                nc.vector.tensor_tensor(
                    out=s3[:], in0=s2[:], in1=bcr[:], op=mybir.AluOpType.add
                )
                od = sb.tile([128, 128], out_dtype, tag="od" + tagsuf, bufs=2)
                nc.scalar.activation(
                    out=od[:], in_=s3[:], func=mybir.ActivationFunctionType.Relu
                )
                return od
            od = sb.tile([128, 128], out_dtype, tag="od" + tagsuf, bufs=2)
            nc.scalar.activation(
                out=od[:],
                in_=agg[:],
                func=mybir.ActivationFunctionType.Relu,
                scale=dinvo[:, t : t + 1],
            )
            return od

        def transpose_to_bf16(src, ident, tagsuf):
            tp = ps.tile([128, 128], src.dtype, tag="tp", bufs=2)
            nc.tensor.transpose(out=tp[:], in_=src[:], identity=ident[:])
            oT = sb.tile([128, 128], BF16, tag="oT" + tagsuf, bufs=2)
            nc.vector.tensor_copy(out=oT[:], in_=tp[:])
            return oT

        def chunk_of(t):
            for j in range(len(sizes)):
                if t < c0s[j + 1]:
                    return j
            raise AssertionError

        # ---- layer 1 agg + gate + layer-2 dense ---------------------------
        dbg_o1 = dbg_o2 = dbg_ht = dbg_p1 = dbg_p2 = None
        if kdbg == "o1":
            dbg_o1 = nc.dram_tensor("dbgo1", [sh_pad, 128], F32, kind="ExternalOutput")
            dbg_o2 = nc.dram_tensor("dbgo2", [sh_pad, 128], F32, kind="ExternalOutput")
        if kdbg == "l2":
            dbg_ht = nc.dram_tensor("dbght", [sh_pad, 128], F32, kind="ExternalOutput")
            dbg_p1 = nc.dram_tensor("dbgp1", [sh_pad, 128], F32, kind="ExternalOutput")
            dbg_p2 = nc.dram_tensor("dbgp2", [sh_pad, 128], F32, kind="ExternalOutput")
        pending = prep_batch(0, (T1f, T1r), "1")
        for g in range(nb):
            fire_batch()
            msgs = pending
            # prep next batch; layer-2 batch 0 must wait until T2's AllGather
            # chunks are EMITTED (Tile deps are emission-ordered), so it is
            # primed after this loop instead.
            if g + 1 < nb:
                pending = prep_batch(g + 1, (T1f, T1r), "1")
            kg = kg_of[g]
            for ti in range(kg):
                t = g * k + ti
                o1 = agg_tile(t, kg, ti, msgs["f"], dstf, dof, bc1r, BF16, "f")
                o2 = agg_tile(t, kg, ti, msgs["r"], dstr, dor, bc1r, BF16, "r")
                if dbg_o1 is not None:
                    o1f = sb.tile([128, 128], F32, tag="o1f", bufs=2)
                    nc.vector.tensor_copy(out=o1f[:], in_=o1[:])
                    nc.sync.dma_start(out=dbg_o1[t * 128 : (t + 1) * 128, :], in_=o1f[:])
                    o2f = sb.tile([128, 128], F32, tag="o2f", bufs=2)
                    nc.vector.tensor_copy(out=o2f[:], in_=o2[:])
                    nc.sync.dma_start(out=dbg_o2[t * 128 : (t + 1) * 128, :], in_=o2f[:])
                o1T = transpose_to_bf16(o1, ident_bf, "1")
                o2T = transpose_to_bf16(o2, ident_bf, "2")
                zps = ps.tile([128, 128], F32, tag="z", bufs=2)
                nc.tensor.matmul(out=zps[:], lhsT=w11Tb[:], rhs=o1T[:], start=True, stop=False)
                nc.tensor.matmul(out=zps[:], lhsT=w12Tb[:], rhs=o2T[:], start=False, stop=True)
                GT = sb.tile([128, 128], BF16, tag="GT", bufs=2)
                nc.scalar.activation(
                    out=GT[:],
                    in_=zps[:],
                    func=mybir.ActivationFunctionType.Sigmoid,
                    bias=b1c[:, :1],
                )
                dT = sb.tile([128, 128], BF16, tag="dT", bufs=2)
                nc.vector.tensor_tensor(
                    out=dT[:], in0=o1T[:], in1=o2T[:], op=mybir.AluOpType.subtract
                )
                pT = sb.tile([128, 128], BF16, tag="pT", bufs=2)
                nc.vector.tensor_tensor(
                    out=pT[:], in0=GT[:], in1=dT[:], op=mybir.AluOpType.mult
                )
                hT = sb.tile([128, 128], BF16, tag="hT", bufs=2)
                nc.vector.tensor_tensor(
                    out=hT[:], in0=pT[:], in1=o2T[:], op=mybir.AluOpType.add
                )
                if kdbg == "l2":
                    htf = sb.tile([128, 128], F32, tag="htf", bufs=2)
                    nc.vector.tensor_copy(out=htf[:], in_=hT[:])
                    nc.sync.dma_start(
                        out=dbg_ht[t * 128 : (t + 1) * 128, :], in_=htf[:]
                    )
                # stage own hT into the AllGather input slab for layer 2
                j = chunk_of(t)
                nc.sync.dma_start(
                    out=HTO[j][:, (t - c0s[j]) * 128 : (t - c0s[j] + 1) * 128],
                    in_=hT[:],
                )

        # hT AllGathers AFTER the agg loop: their data deps (HTO writes of the
        # last batch) transitively order them after every gather DMA drained —
        # a collective executing while SWDGE gathers are in flight corrupts
        # data (observed as flaky NaN / device faults).
        for j in range(len(sizes)):
            ag_chunk(j)

        # layer-2 dense: all tiles from the AllGathered hT (lhsT directly)
        for j in range(len(sizes)):
            nj = sizes[j]
            for r in range(w):
                g0 = reg_off_rows[j] // 128 + r * nj
                done = 0
                while done < nj:
                    nt = min(8, nj - done)
                    hs = sb.tile([128, nt * 128], BF16, tag="xa", bufs=3)
                    nc.sync.dma_start(
                        out=hs[:],
                        in_=HTF[j][r, :, done * 128 : (done + nt) * 128],
                    )
                    dense_slab(g0 + done, nt, hs, W2b, T2f, T2r)
                    done += nt

        if kdbg == "l2":
            dbg_t20 = nc.dram_tensor("dbgt20", [r_split, 128], BF16, kind="ExternalOutput")
            dbg_t21 = nc.dram_tensor(
                "dbgt21", [np_pad - r_split, 128], BF16, kind="ExternalOutput"
            )
            nc.sync.dma_start(out=dbg_t20[:], in_=T2f[0][:])
            nc.sync.dma_start(out=dbg_t21[:], in_=T2f[1][:])

        # ---- layer 2 agg + gate + output ---------------------------------
        pending = prep_batch(0, (T2f, T2r), "2")
        for g in range(nb):
            fire_batch()
            msgs = pending
            if g + 1 < nb:
                pending = prep_batch(g + 1, (T2f, T2r), "2")
            kg = kg_of[g]
            for ti in range(kg):
                t = g * k + ti
                p1 = agg_tile(t, kg, ti, msgs["f"], dstf, dof, bc2r, F32, "f")
                p2 = agg_tile(t, kg, ti, msgs["r"], dstr, dor, bc2r, F32, "r")
                if dbg_p1 is not None:
                    nc.sync.dma_start(out=dbg_p1[t * 128 : (t + 1) * 128, :], in_=p1[:])
                    nc.sync.dma_start(out=dbg_p2[t * 128 : (t + 1) * 128, :], in_=p2[:])
                p1T = transpose_to_bf16(p1, ident_f32, "1")
                p2T = transpose_to_bf16(p2, ident_f32, "2")
                zps = ps.tile([128, 128], F32, tag="z", bufs=2)
                nc.tensor.matmul(out=zps[:], lhsT=w21Tb[:], rhs=p1T[:], start=True, stop=False)
                nc.tensor.matmul(out=zps[:], lhsT=w22Tb[:], rhs=p2T[:], start=False, stop=True)
                G2T = sb.tile([128, 128], BF16, tag="GT", bufs=2)
                nc.scalar.activation(
                    out=G2T[:],
                    in_=zps[:],
                    func=mybir.ActivationFunctionType.Sigmoid,
                    bias=b2c[:, :1],
                )
                g2p = ps.tile([128, 128], BF16, tag="tp", bufs=2)
                nc.tensor.transpose(out=g2p[:], in_=G2T[:], identity=ident_bf[:])
                g2s = sb.tile([128, 128], F32, tag="g2s", bufs=2)
                nc.vector.tensor_copy(out=g2s[:], in_=g2p[:])
                dd = sb.tile([128, 128], F32, tag="dd", bufs=2)
                nc.vector.tensor_tensor(
                    out=dd[:], in0=p1[:], in1=p2[:], op=mybir.AluOpType.subtract
                )
                pr = sb.tile([128, 128], F32, tag="pr", bufs=2)
                nc.vector.tensor_tensor(
                    out=pr[:], in0=dd[:], in1=g2s[:], op=mybir.AluOpType.mult
                )
                ot = sb.tile([128, 128], F32, tag="ot", bufs=2)
                nc.vector.tensor_tensor(
                    out=ot[:], in0=pr[:], in1=p2[:], op=mybir.AluOpType.add
                )
                nc.sync.dma_start(out=out_d[t * 128 : (t + 1) * 128, :], in_=ot[:])

    nc.compile()
    return nc


# ---------------------------------------------------------------------------
# full pipeline
# ---------------------------------------------------------------------------


def make_in_maps(inputs, meta, tables):
    import ml_dtypes

    w = W_CORES
    sh_real, sh_pad = meta["sh_real"], meta["sh_pad"]
    t_own, np_pad = meta["t_own"], meta["np_pad"]
    n_real = w * sh_real
    slot_of = meta["slot_of"]

    x = np.asarray(inputs["x"], np.float32)
    slots = slot_of(np.arange(n_real))
    x_slot = np.zeros((np_pad, D), np.float32)
    x_slot[slots] = x
    xT = np.ascontiguousarray(x_slot.T).astype(ml_dtypes.bfloat16)

    def t2(a):
        return np.ascontiguousarray(np.asarray(a, np.float32).T)

    W1 = np.asarray(inputs["W1"], np.float32)
    W2 = np.asarray(inputs["W2"], np.float32)
    b1c = np.asarray(inputs["b1"], np.float32).reshape(128, 1)
    b2c = np.asarray(inputs["b2"], np.float32).reshape(128, 1)
    bc1r = np.broadcast_to(np.asarray(inputs["bc1"], np.float32), (128, 128)).copy()
    bc2r = np.broadcast_to(np.asarray(inputs["bc2"], np.float32), (128, 128)).copy()

    in_maps = []
    for c in range(w):
        in_maps.append(
            dict(
                xT=xT,
                W1=W1, W2=W2,
                w11T=t2(inputs["w11"]), w12T=t2(inputs["w12"]),
                w21T=t2(inputs["w21"]), w22T=t2(inputs["w22"]),
                b1c=b1c, b2c=b2c, bc1r=bc1r, bc2r=bc2r,
                dof=tables["dof"][c], dor=tables["dor"][c],
                dallf=tables["dallf"], dallr=tables["dallr"],
                ilof=tables["ilo_f"][c], ihif=tables["ihi_f"][c],
                ilor=tables["ilo_r"][c], ihir=tables["ihi_r"][c],
                dstf=tables["dst_f"][c], dstr=tables["dst_r"][c],
            )
        )
    return in_maps


def assemble_output(results, meta):
    sh_real, sh_pad = meta["sh_real"], meta["sh_pad"]
    n_real = W_CORES * sh_real
    full = np.concatenate([r["out"] for r in results], axis=0)
    v = np.arange(n_real)
    rows = (v // sh_real) * sh_pad + (v % sh_real)
    return np.ascontiguousarray(full[rows]).astype(np.float32)


_CACHE = {}


def _get_program(meta, add_bc1, add_bc2):
    key = (meta["t_own"], meta["b_lo"], meta["b_hi"], meta["k"], add_bc1, add_bc2)
    if key not in _CACHE:
        _CACHE[key] = build_program(
            meta["t_own"], meta["b_lo"], meta["b_hi"], meta["k"],
            add_bc1=add_bc1, add_bc2=add_bc2,
        )
    return _CACHE[key]


def _install_ntff_hook():
    """Shim antenv.axon_hooks (absent in this image) so run_bass_kernel_spmd
    trace=True can capture NTFF profiles via libaxon_pjrt.so ctypes calls."""
    import contextlib
    import ctypes
    import types

    if "antenv.axon_hooks" in sys.modules:
        return
    so_path = "/opt/axon/libaxon_pjrt.so"
    holder = {}
    m = types.ModuleType("antenv.axon_hooks")
    m.set_axon_ntff_profile_hook = lambda h: holder.__setitem__("h", h)
    m.get_axon_ntff_profile_hook = lambda: holder.get("h")
    sys.modules["antenv.axon_hooks"] = m
    try:
        import antenv

        antenv.axon_hooks = m
    except ImportError:
        pass
    try:
        lib = ctypes.CDLL(so_path)
    except OSError:
        return
    if not hasattr(lib, "axon_start_nrt_profile"):
        return
    lib.axon_start_nrt_profile.argtypes = [
        ctypes.POINTER(ctypes.c_int64),
        ctypes.c_size_t,
    ]
    lib.axon_start_nrt_profile.restype = ctypes.c_int64
    lib.axon_stop_nrt_profile.argtypes = [ctypes.c_char_p]
    lib.axon_stop_nrt_profile.restype = ctypes.c_int64

    @contextlib.contextmanager
    def _hook(output_dir, device_ids):
        import jax

        jax.devices()
        if device_ids:
            ids = (ctypes.c_int64 * len(device_ids))(*device_ids)
            rc = lib.axon_start_nrt_profile(ids, len(device_ids))
        else:
            rc = lib.axon_start_nrt_profile(None, 0)
        if rc != 0:
            raise RuntimeError(f"axon_start_nrt_profile rc={rc}")
        try:
            yield
        finally:
            n = lib.axon_stop_nrt_profile(str(output_dir).encode())
            print(f"profile: {n} file(s) written to {output_dir}", file=sys.stderr)

    holder["h"] = _hook


def _patch_upload_artifacts():
    import concourse.bass_utils as bu

    bu.upload_artifacts = lambda tmpdir: tmpdir


def kernel(**inputs):
    x = np.asarray(inputs["x"], np.float32)
    n_real = x.shape[0]
    meta, tables = host_prepare(x, np.asarray(inputs["edge_index"]), n_real)
    add_bc1 = bool(np.any(np.asarray(inputs["bc1"]) != 0))
    add_bc2 = bool(np.any(np.asarray(inputs["bc2"]) != 0))
    nc = _get_program(meta, add_bc1, add_bc2)
    in_maps = make_in_maps(inputs, meta, tables)
    if bool(int(os.environ.get("KERNEL_TRACE", "0"))):
        _install_ntff_hook()
        _patch_upload_artifacts()
    res = run_bass_kernel_spmd(
        nc,
        in_maps,
        core_ids=list(range(W_CORES)),
        trace=bool(int(os.environ.get("KERNEL_TRACE", "0"))),
    )
    global LAST_EXEC_NS
    LAST_EXEC_NS = res.exec_time_ns
    if res.exec_time_ns is not None:
        print(f"HW exec time: {res.exec_time_ns} ns")
    return assemble_output(res.results, meta)


LAST_EXEC_NS = None


# revision 31
# speedup vs baseline: 1.3055x; 1.0232x over previous
"""Trainium2 Bass kernel for a 2-layer dual-direction gated GCN (DGGCN).

Contract: kernel(**inputs) takes the FULL unsharded inputs (as produced by
setup_inputs) and returns the FULL [N, D] float32 output.

Strategy (8 NeuronCores, node partition):
  - Nodes are remapped into a chunk-major padded slot space: the node axis is
    split into ag_chunks regions; within a region, cores' tiles are laid out
    contiguously per core so a chunked AllGather writes each region with one
    contiguous collective. Self-loops are explicit (v, v) edges. Edges are
    bucketed by destination tile (forward) / source tile (reverse) and split
    per tile into lo/hi halves of the slot space (region boundary) so gather
    indices fit dma_gather's int16 format. Uniform B_LO/B_HI block counts per
    tile keep the program SPMD.
  - Per layer each core computes h @ W for OWN tiles only, writes its slab,
    and a chunked AllGather replicates the bf16 message table to all cores.
    dinv[src] is NOT folded into the table; instead it is folded into the
    one-hot scatter matrices, so one table serves both edge directions.
  - Aggregation per own dst tile: batched dma_gather of table rows. All
    gathers are issued as prepare_only descriptor-generation (GPSIMD) one
    batch ahead and fired with trigger_dma, so SWDGE generation overlaps
    compute/DMA. Scatter matrices are built on-chip with a fused
    tensor_scalar (iota == dst_code) * dinv_src, then PE matmuls accumulate
    segment sums in PSUM; relu(x)*dinv_dst is fused on ACT.
"""

import os
import sys

sys.path.insert(0, "/opt/trn_rl_repo")

import numpy as np

import concourse.bacc as bacc
import concourse.bass as bass
import concourse.tile as tile
from concourse import mybir
from concourse.bass_utils import run_bass_kernel_spmd
from concourse.masks import make_identity

F32 = mybir.dt.float32
BF16 = mybir.dt.bfloat16
I32 = mybir.dt.int32
I16 = mybir.dt.int16

W_CORES = 8
D = 128
PAD_DST = 200.0  # sentinel local-dst id (never matches iota 0..127)
AG_CHUNKS = 2


# ---------------------------------------------------------------------------
# host-side graph preprocessing (index bucketing / sharding metadata only)
# ---------------------------------------------------------------------------


def _chunk_sizes(t_own):
    sizes = []
    base = 0
    for j in range(AG_CHUNKS):
        n = (t_own - base) // (AG_CHUNKS - j)
        if n > 0:
            sizes.append(n)
            base += n
    return sizes


def _pack16(flat):
    """Pack an int array [n] into dma_gather's [128, n//16] int16 layout:
    index i lives at partition i%16, column i//16, replicated to 8 stripes."""
    n = flat.shape[0]
    assert n % 16 == 0
    return np.tile(flat.reshape(n // 16, 16).T, (8, 1))


def host_prepare(x, edge_index, n_real):
    w = W_CORES
    assert n_real % w == 0
    sh_real = n_real // w
    t_own = (sh_real + 127) // 128
    sh_pad = t_own * 128
    t_all = w * t_own
    np_pad = t_all * 128
    sizes = _chunk_sizes(t_own)
    # chunk-major region starts (rows) and per-core tile starts within chunks
    reg_rows = [w * n * 128 for n in sizes]
    reg_off = np.concatenate([[0], np.cumsum(reg_rows)]).astype(np.int64)
    r_split = int(reg_off[1])  # lo/hi gather split at region-0 boundary
    assert r_split <= 32767 + 1 and (np_pad - r_split) <= 32767 + 1
    chunk_of_tile = np.concatenate(
        [np.full(n, j, np.int64) for j, n in enumerate(sizes)]
    )
    tile_in_chunk = np.concatenate([np.arange(n) for n in sizes])
    c0_of_chunk = np.concatenate([[0], np.cumsum(sizes)]).astype(np.int64)

    sizes_arr = np.asarray(sizes, np.int64)

    def slot_of(v):
        c = v // sh_real
        l = v % sh_real
        t = l // 128
        off = l % 128
        j = chunk_of_tile[t]
        return reg_off[j] + (c * sizes_arr[j] + tile_in_chunk[t]) * 128 + off

    src = np.asarray(edge_index[0], dtype=np.int64)
    dst = np.asarray(edge_index[1], dtype=np.int64)
    ss = slot_of(src)
    ds = slot_of(dst)
    selfs = slot_of(np.arange(n_real, dtype=np.int64))

    # degrees (+1 self-loop) in slot space; dinv on host
    deg_f = np.ones(np_pad, np.float32)
    deg_r = np.ones(np_pad, np.float32)
    np.add.at(deg_f, ds, 1.0)
    np.add.at(deg_r, ss, 1.0)
    dinv_f = (1.0 / np.sqrt(deg_f)).astype(np.float32)
    dinv_r = (1.0 / np.sqrt(deg_r)).astype(np.float32)

    # global tile id of a slot (for bucketing): region-aware
    def tile_of_slot(s):
        out = np.empty(s.shape, np.int64)
        for j in range(len(sizes)):
            m = (s >= reg_off[j]) & (s < reg_off[j + 1])
            rel = (s[m] - reg_off[j]) // 128  # core-major within region
            c = rel // sizes[j]
            t = rel % sizes[j]
            out[m] = c * t_own + c0_of_chunk[j] + t
        return out

    # self-loops as explicit edges
    agg_f = np.concatenate([ds, selfs])
    gat_f = np.concatenate([ss, selfs])
    agg_r = np.concatenate([ss, selfs])
    gat_r = np.concatenate([ds, selfs])

    def bucket(agg_slot, gather_slot):
        tile_id = tile_of_slot(agg_slot)
        hi = (gather_slot >= r_split).astype(np.int64)
        order = np.lexsort((hi, tile_id))
        t_s = tile_id[order]
        g_s = gather_slot[order]
        h_s = hi[order]
        loc_s = (agg_slot[order] % 128).astype(np.float32)
        n_lo = np.bincount(t_s[h_s == 0], minlength=t_all)
        n_hi = np.bincount(t_s[h_s == 1], minlength=t_all)
        return t_s, g_s, h_s, loc_s, n_lo, n_hi

    bf = bucket(agg_f, gat_f)
    br = bucket(agg_r, gat_r)
    b_lo = int(max(bf[4].max(), br[4].max()) + 127) // 128
    b_hi = int(max(bf[5].max(), br[5].max()) + 127) // 128
    bt = b_lo + b_hi

    def build_tables(t_s, g_s, h_s, loc_s, n_lo, n_hi):
        idx_lo = np.zeros((t_all, b_lo * 128), np.int16)
        idx_hi = np.zeros((t_all, b_hi * 128), np.int16)
        dst_t = np.full((t_all, bt * 128), PAD_DST, np.float32)
        n_edges = len(t_s)
        tile_starts = np.zeros(t_all + 1, np.int64)
        np.cumsum(n_lo + n_hi, out=tile_starts[1:])
        pos_in_tile = np.arange(n_edges) - tile_starts[t_s]
        pos_lo = pos_in_tile
        pos_hi = pos_in_tile - n_lo[t_s]
        mlo = h_s == 0
        mhi = h_s == 1
        idx_lo[t_s[mlo], pos_lo[mlo]] = g_s[mlo].astype(np.int16)
        idx_hi[t_s[mhi], pos_hi[mhi]] = (g_s[mhi] - r_split).astype(np.int16)
        dst_t[t_s[mlo], pos_lo[mlo]] = loc_s[mlo]
        dst_t[t_s[mhi], b_lo * 128 + pos_hi[mhi]] = loc_s[mhi]
        return idx_lo, idx_hi, dst_t

    tbl_f = build_tables(*bf)
    tbl_r = build_tables(*br)

    k = min(3, t_own)
    nb = (t_own + k - 1) // k

    def per_core(idx_lo, idx_hi, dst_t):
        import ml_dtypes
        idxlo_l, idxhi_l, dst_l = [], [], []
        for c in range(w):
            sl = slice(c * t_own, (c + 1) * t_own)
            ilo = idx_lo[sl]
            ihi = idx_hi[sl]
            plo = np.concatenate(
                [_pack16(ilo[g * k : min((g + 1) * k, t_own)].reshape(-1)) for g in range(nb)],
                axis=1,
            )
            phi = np.concatenate(
                [_pack16(ihi[g * k : min((g + 1) * k, t_own)].reshape(-1)) for g in range(nb)],
                axis=1,
            )

            def t_pack(a):
                return np.ascontiguousarray(
                    a.reshape(t_own, bt, 128).transpose(2, 0, 1).reshape(128, t_own * bt)
                ).astype(ml_dtypes.bfloat16)

            idxlo_l.append(np.ascontiguousarray(plo))
            idxhi_l.append(np.ascontiguousarray(phi))
            dst_l.append(t_pack(dst_t[sl]))
        return idxlo_l, idxhi_l, dst_l

    ilo_f, ihi_f, dst_f = per_core(*tbl_f)
    ilo_r, ihi_r, dst_r = per_core(*tbl_r)

    # own-tile dinv columns [128, t_own] per core per direction
    def own_dinv(dinv):
        cols = []
        for c in range(w):
            rows = np.empty(sh_pad, np.float32)
            for t in range(t_own):
                j = chunk_of_tile[t]
                r0 = reg_off[j] + (c * sizes[j] + tile_in_chunk[t]) * 128
                rows[t * 128 : (t + 1) * 128] = dinv[r0 : r0 + 128]
            cols.append(np.ascontiguousarray(rows.reshape(t_own, 128).T))
        return cols

    dof = own_dinv(dinv_f)
    dor = own_dinv(dinv_r)

    meta = dict(
        sh_real=sh_real, sh_pad=sh_pad, t_own=t_own, t_all=t_all, np_pad=np_pad,
        b_lo=b_lo, b_hi=b_hi, k=k, nb=nb, sizes=sizes, r_split=r_split,
        slot_of=slot_of,
    )
    tables = dict(
        ilo_f=ilo_f, ihi_f=ihi_f, dst_f=dst_f,
        ilo_r=ilo_r, ihi_r=ihi_r, dst_r=dst_r,
        dof=dof, dor=dor,
        dallf=np.ascontiguousarray(dinv_f.reshape(t_all, 128).T),
        dallr=np.ascontiguousarray(dinv_r.reshape(t_all, 128).T),
    )
    return meta, tables


# ---------------------------------------------------------------------------
# device program
# ---------------------------------------------------------------------------


def build_program(t_own, b_lo, b_hi, k, add_bc1=False, add_bc2=False):
    w = W_CORES
    t_all = w * t_own
    np_pad = t_all * 128
    sh_pad = t_own * 128
    bt = b_lo + b_hi
    nb = (t_own + k - 1) // k
    kg_of = [min(k, t_own - g * k) for g in range(nb)]
    lo_off = [sum(kg_of[:g]) * b_lo * 8 for g in range(nb + 1)]
    hi_off = [sum(kg_of[:g]) * b_hi * 8 for g in range(nb + 1)]
    lo_cols = lo_off[-1]
    hi_cols = hi_off[-1]
    sizes = _chunk_sizes(t_own)
    c0s = [0]
    for n in sizes:
        c0s.append(c0s[-1] + n)
    reg_off_rows = [0]
    for n in sizes:
        reg_off_rows.append(reg_off_rows[-1] + w * n * 128)
    r_split = reg_off_rows[1]
    assert len(sizes) == 2  # lo/hi gather tables == the two AG chunk tensors

    nc = bacc.Bacc(
        "TRN2", target_bir_lowering=False, debug=False, num_devices=w,
        num_swdge_queues=4,
    )

    # ---- external I/O -----------------------------------------------------
    xT_d = nc.dram_tensor("xT", [128, np_pad], BF16, kind="ExternalInput")
    W1_d = nc.dram_tensor("W1", [128, 128], F32, kind="ExternalInput")
    W2_d = nc.dram_tensor("W2", [128, 128], F32, kind="ExternalInput")
    w11T_d = nc.dram_tensor("w11T", [128, 128], F32, kind="ExternalInput")
    w12T_d = nc.dram_tensor("w12T", [128, 128], F32, kind="ExternalInput")
    w21T_d = nc.dram_tensor("w21T", [128, 128], F32, kind="ExternalInput")
    w22T_d = nc.dram_tensor("w22T", [128, 128], F32, kind="ExternalInput")
    b1c_d = nc.dram_tensor("b1c", [128, 1], F32, kind="ExternalInput")
    b2c_d = nc.dram_tensor("b2c", [128, 1], F32, kind="ExternalInput")
    bc1r_d = nc.dram_tensor("bc1r", [128, 128], F32, kind="ExternalInput")
    bc2r_d = nc.dram_tensor("bc2r", [128, 128], F32, kind="ExternalInput")
    dof_d = nc.dram_tensor("dof", [128, t_own], F32, kind="ExternalInput")
    dor_d = nc.dram_tensor("dor", [128, t_own], F32, kind="ExternalInput")
    dallf_d = nc.dram_tensor("dallf", [128, t_all], F32, kind="ExternalInput")
    dallr_d = nc.dram_tensor("dallr", [128, t_all], F32, kind="ExternalInput")
    ilof_d = nc.dram_tensor("ilof", [128, lo_cols], I16, kind="ExternalInput")
    ihif_d = nc.dram_tensor("ihif", [128, hi_cols], I16, kind="ExternalInput")
    ilor_d = nc.dram_tensor("ilor", [128, lo_cols], I16, kind="ExternalInput")
    ihir_d = nc.dram_tensor("ihir", [128, hi_cols], I16, kind="ExternalInput")
    dstf_d = nc.dram_tensor("dstf", [128, t_own * bt], BF16, kind="ExternalInput")
    dstr_d = nc.dram_tensor("dstr", [128, t_own * bt], BF16, kind="ExternalInput")
    out_d = nc.dram_tensor("out", [sh_pad, 128], F32, kind="ExternalOutput")

    from contextlib import ExitStack

    with tile.TileContext(nc) as tc, ExitStack() as ctx:
        sb = ctx.enter_context(tc.tile_pool(name="sb", bufs=1))
        ps = ctx.enter_context(tc.tile_pool(name="ps", bufs=1, space="PSUM"))
        dr = ctx.enter_context(tc.tile_pool(name="dr", bufs=1, space="DRAM"))

        # message tables: per (layer, direction) pre-scaled by dinv_dir[u],
        # computed REPLICATED on every core (no collective). Separate lo/hi
        # tensors so lo gathers only depend on the first half of the dense
        # pass (subrange deps are tensor-granular).
        def table(name):
            return (
                dr.tile([r_split, 128], BF16, name=name + "lo"),
                dr.tile([np_pad - r_split, 128], BF16, name=name + "hi"),
            )

        T1f, T1r = table("T1f"), table("T1r")
        T2f, T2r = table("T2f"), table("T2r")
        # gated layer-1 output hT: own chunk slabs -> AllGathered (only coll.)
        HTO = [
            dr.tile([128, n * 128], BF16, name=f"HTO{j}")
            for j, n in enumerate(sizes)
        ]
        HTF = [
            dr.tile([w, 128, n * 128], BF16, name=f"HTF{j}", addr_space="Shared")
            for j, n in enumerate(sizes)
        ]

        # ---- constants / small persistent SBUF ----
        def load_const(dram, shape, dtype=F32, name=None):
            t = sb.tile(shape, dtype, name=name or dram.name + "_sb")
            nc.sync.dma_start(out=t[:], in_=dram[:])
            return t

        def load_cast_bf16(dram, name):
            t32 = sb.tile([128, 128], F32, name=name + "_f32")
            nc.sync.dma_start(out=t32[:], in_=dram[:])
            t16 = sb.tile([128, 128], BF16, name=name)
            nc.vector.tensor_copy(out=t16[:], in_=t32[:])
            return t16

        W1b = load_cast_bf16(W1_d, "W1b")
        W2b = load_cast_bf16(W2_d, "W2b")
        w11Tb = load_cast_bf16(w11T_d, "w11Tb")
        w12Tb = load_cast_bf16(w12T_d, "w12Tb")
        w21Tb = load_cast_bf16(w21T_d, "w21Tb")
        w22Tb = load_cast_bf16(w22T_d, "w22Tb")
        b1c = load_const(b1c_d, [128, 1], name="b1c")
        b2c = load_const(b2c_d, [128, 1], name="b2c")
        bc1r = load_const(bc1r_d, [128, 128], name="bc1r") if add_bc1 else None
        bc2r = load_const(bc2r_d, [128, 128], name="bc2r") if add_bc2 else None
        dof = load_const(dof_d, [128, t_own], name="dof")
        dor = load_const(dor_d, [128, t_own], name="dor")
        dstf = load_const(dstf_d, [128, t_own * bt], BF16, "dstf_sb")
        dstr = load_const(dstr_d, [128, t_own * bt], BF16, "dstr_sb")
        dallf = load_const(dallf_d, [128, t_all], F32, "dallf_sb")
        dallr = load_const(dallr_d, [128, t_all], F32, "dallr_sb")

        iota_i = sb.tile([128, 128], I32, name="iota_i")
        nc.gpsimd.iota(iota_i[:], pattern=[[1, 128]], base=0, channel_multiplier=0)
        iota_bf = sb.tile([128, 128], BF16, name="iota_bf")
        nc.vector.tensor_copy(out=iota_bf[:], in_=iota_i[:])
        # contiguous bt-times-repeated iota, so the per-tile IS_EQ reads a
        # non-broadcast in0 (broadcast reads force the slow 1-elem/cycle path)
        iota_full = sb.tile([128, bt, 128], BF16, name="iota_full")
        nc.vector.tensor_copy(
            out=iota_full[:],
            in_=iota_bf[:]
            .rearrange("p (o d) -> p o d", o=1)
            .to_broadcast([128, bt, 128]),
        )

        ident_bf = sb.tile([128, 128], BF16, name="ident_bf")
        make_identity(nc, ident_bf[:])
        ident_f32 = sb.tile([128, 128], F32, name="ident_f32")
        make_identity(nc, ident_f32[:])

        # own tile t -> column offset in xT (chunk-major slot space)
        def own_col(t):
            for j in range(len(sizes)):
                if t < c0s[j + 1]:
                    # column index c is baked per-core via per-core input? No:
                    # xT is the FULL table; own columns depend on core id.
                    raise AssertionError
            raise AssertionError

        # NOTE: xT is full; each core must read its own columns, which depend
        # on the core id. SPMD programs are identical across cores, so we pass
        # the own x slab as a separate per-core input instead.
        # (own_col unused; kept for clarity)

        # ---- dense pass: replicated over ALL tiles, slab-based ------------
        half_t = r_split // 128

        def dense_slab(g0, nt, xs_bf, Wb, Tf, Tr):
            """xs_bf: [128 d, nt*128 n] lhsT slab for tiles g0..g0+nt; writes
            dinv-scaled rows of the forward/reverse tables (one DMA each)."""
            hfs = sb.tile([128, nt * 128], BF16, tag="hfs", bufs=3)
            hrs = sb.tile([128, nt * 128], BF16, tag="hrs", bufs=3)
            for i in range(nt):
                g = g0 + i
                ph = ps.tile([128, 128], F32, tag="phd", bufs=2)
                nc.tensor.matmul(
                    out=ph[:],
                    lhsT=xs_bf[:, i * 128 : (i + 1) * 128],
                    rhs=Wb[:],
                    start=True,
                    stop=True,
                )
                nc.vector.tensor_scalar(
                    out=hfs[:, i * 128 : (i + 1) * 128],
                    in0=ph[:],
                    scalar1=dallf[:, g : g + 1],
                    scalar2=None,
                    op0=mybir.AluOpType.mult,
                )
                nc.scalar.activation(
                    out=hrs[:, i * 128 : (i + 1) * 128],
                    in_=ph[:],
                    func=mybir.ActivationFunctionType.Copy,
                    scale=dallr[:, g : g + 1],
                )
            hx = 0 if g0 < half_t else 1
            r0 = (g0 - hx * half_t) * 128
            for tbl, slb in ((Tf[hx], hfs), (Tr[hx], hrs)):
                nc.sync.dma_start(
                    out=tbl[r0 : r0 + nt * 128, :].rearrange("(t p) d -> p t d", p=128),
                    in_=slb[:].rearrange("p (t d) -> p t d", t=nt),
                )

        def ag_chunk(j):
            nc.gpsimd.collective_compute(
                "AllGather",
                mybir.AluOpType.bypass,
                replica_groups=[list(range(w))],
                ins=[HTO[j].opt()],
                outs=[HTF[j].opt()],
            )

        # layer-1 dense: all tiles from the replicated x (slabs never cross
        # the lo/hi boundary: half_t % slab == 0 enforced by slab choice)
        slab = 1
        for cand in range(min(8, half_t), 0, -1):
            if half_t % cand == 0 and (t_all - half_t) % cand == 0:
                slab = cand
                break
        for g0 in range(0, t_all, slab):
            xs = sb.tile([128, slab * 128], BF16, tag="xa", bufs=3)
            nc.sync.dma_start(
                out=xs[:], in_=xT_d[:, g0 * 128 : (g0 + slab) * 128]
            )
            dense_slab(g0, slab, xs, W1b, T1f, T1r)

        kdbg = os.environ.get("KDBG", "")
        if kdbg == "t1":
            dbg0 = nc.dram_tensor("dbg0", [r_split, 128], BF16, kind="ExternalOutput")
            dbg1 = nc.dram_tensor(
                "dbg1", [np_pad - r_split, 128], BF16, kind="ExternalOutput"
            )
            nc.sync.dma_start(out=dbg0[:], in_=T1f[0][:])
            nc.sync.dma_start(out=dbg1[:], in_=T1f[1][:])

        # ---- gather machinery --------------------------------------------
        gq_sem = [nc.alloc_semaphore(f"gq{q}") for q in range(4)]

        idx_src = dict(f=(ilof_d, ihif_d), r=(ilor_d, ihir_d))

        use_prep = bool(int(os.environ.get("KPREP", "0")))

        def prep_batch(g, Tpair, tag):
            """Emit idx loads + gathers for batch g, both dirs.
            Returns dict dir -> msg tile."""
            kg = kg_of[g]
            msgs = {}
            for di, dname in enumerate("fr"):
                T = Tpair[di]
                lo_d, hi_d = idx_src[dname]
                msg = sb.tile([128, kg * bt, 128], BF16, tag=f"msg{dname}", bufs=3)
                ilo = sb.tile([128, kg * b_lo * 8], I16, tag=f"ilo{dname}", bufs=3)
                nc.sync.dma_start(out=ilo[:], in_=lo_d[:, lo_off[g] : lo_off[g + 1]])
                ihi = sb.tile([128, kg * b_hi * 8], I16, tag=f"ihi{dname}", bufs=3)
                nc.sync.dma_start(out=ihi[:], in_=hi_d[:, hi_off[g] : hi_off[g + 1]])
                q_lo = di * 2
                q_hi = di * 2 + 1
                pk = dict(prepare_only=True) if use_prep else {}
                nc.gpsimd.dma_gather(
                    out_ap=msg[:, : kg * b_lo, :],
                    in_ap=T[0][:],
                    idxs_ap=ilo[:],
                    num_idxs=kg * b_lo * 128,
                    num_idxs_reg=kg * b_lo * 128,
                    elem_size=128,
                    single_packet=False,
                    queue_num=q_lo,
                    sem=gq_sem[q_lo] if use_prep else None,
                    **pk,
                )
                nc.gpsimd.dma_gather(
                    out_ap=msg[:, kg * b_lo :, :],
                    in_ap=T[1][:],
                    idxs_ap=ihi[:],
                    num_idxs=kg * b_hi * 128,
                    num_idxs_reg=kg * b_hi * 128,
                    elem_size=128,
                    single_packet=False,
                    queue_num=q_hi,
                    sem=gq_sem[q_hi] if use_prep else None,
                    **pk,
                )
                msgs[dname] = msg
            return msgs

        def fire_batch():
            if use_prep:
                for q in range(4):
                    nc.gpsimd.trigger_dma(count=None, queue_num=q)

        def msg_block(msg, kg, ti, b):
            if b < b_lo:
                return msg[:, ti * b_lo + b, :]
            return msg[:, kg * b_lo + ti * b_hi + (b - b_lo), :]

        def agg_tile(t, kg, ti, msg, dst_sb, dinvo, bcr, out_dtype, tagsuf):
            # one-hot S[e, b, j] = (iota j == dst_code[e, b])
            S = sb.tile([128, bt, 128], BF16, tag="S" + tagsuf, bufs=3)
            nc.vector.tensor_tensor(
                out=S[:],
                in0=iota_full[:],
                in1=dst_sb[:, t * bt : (t + 1) * bt].to_broadcast([128, bt, 128]),
                op=mybir.AluOpType.is_equal,
            )
            agg = ps.tile([128, 128], F32, tag="agg", bufs=2)
            for b in range(bt):
                nc.tensor.matmul(
                    out=agg[:],
                    lhsT=S[:, b, :],
                    rhs=msg_block(msg, kg, ti, b),
                    start=(b == 0),
                    stop=(b == bt - 1),
                )
            if bcr is not None:
                s2 = sb.tile([128, 128], F32, tag="s2" + tagsuf, bufs=2)
                nc.vector.tensor_scalar(
                    out=s2[:],
                    in0=agg[:],
                    scalar1=dinvo[:, t : t + 1],
                    scalar2=None,
                    op0=mybir.AluOpType.mult,
                )
                s3 = sb.tile([128, 128], F32, tag="s3" + tagsuf, bufs=2)
                nc.vector.tensor_tensor(
                    out=s3[:], in0=s2[:], in1=bcr[:], op=mybir.AluOpType.add
                )
                od = sb.tile([128, 128], out_dtype, tag="od" + tagsuf, bufs=2)
                nc.scalar.activation(
                    out=od[:], in_=s3[:], func=mybir.ActivationFunctionType.Relu
                )
                return od
            od = sb.tile([128, 128], out_dtype, tag="od" + tagsuf, bufs=2)
            nc.scalar.activation(
                out=od[:],
                in_=agg[:],
                func=mybir.ActivationFunctionType.Relu,
                scale=dinvo[:, t : t + 1],
            )
            return od

        def transpose_to_bf16(src, ident, tagsuf):
            tp = ps.tile([128, 128], src.dtype, tag="tp", bufs=2)
            nc.tensor.transpose(out=tp[:], in_=src[:], identity=ident[:])
            oT = sb.tile([128, 128], BF16, tag="oT" + tagsuf, bufs=2)
            nc.vector.tensor_copy(out=oT[:], in_=tp[:])
            return oT

        def chunk_of(t):
            for j in range(len(sizes)):
                if t < c0s[j + 1]:
                    return j
            raise AssertionError

        # ---- layer 1 agg + gate + layer-2 dense ---------------------------
        dbg_o1 = dbg_o2 = dbg_ht = dbg_p1 = dbg_p2 = None
        if kdbg == "o1":
            dbg_o1 = nc.dram_tensor("dbgo1", [sh_pad, 128], F32, kind="ExternalOutput")
            dbg_o2 = nc.dram_tensor("dbgo2", [sh_pad, 128], F32, kind="ExternalOutput")
        if kdbg == "l2":
            dbg_ht = nc.dram_tensor("dbght", [sh_pad, 128], F32, kind="ExternalOutput")
            dbg_p1 = nc.dram_tensor("dbgp1", [sh_pad, 128], F32, kind="ExternalOutput")
            dbg_p2 = nc.dram_tensor("dbgp2", [sh_pad, 128], F32, kind="ExternalOutput")
        pending = prep_batch(0, (T1f, T1r), "1")
        for g in range(nb):
            fire_batch()
            msgs = pending
            # prep next batch; layer-2 batch 0 must wait until T2's AllGather
            # chunks are EMITTED (Tile deps are emission-ordered), so it is
            # primed after this loop instead.
            if g + 1 < nb:
                pending = prep_batch(g + 1, (T1f, T1r), "1")
            kg = kg_of[g]
            for ti in range(kg):
                t = g * k + ti
                o1 = agg_tile(t, kg, ti, msgs["f"], dstf, dof, bc1r, BF16, "f")
                o2 = agg_tile(t, kg, ti, msgs["r"], dstr, dor, bc1r, BF16, "r")
                if dbg_o1 is not None:
                    o1f = sb.tile([128, 128], F32, tag="o1f", bufs=2)
                    nc.vector.tensor_copy(out=o1f[:], in_=o1[:])
                    nc.sync.dma_start(out=dbg_o1[t * 128 : (t + 1) * 128, :], in_=o1f[:])
                    o2f = sb.tile([128, 128], F32, tag="o2f", bufs=2)
                    nc.vector.tensor_copy(out=o2f[:], in_=o2[:])
                    nc.sync.dma_start(out=dbg_o2[t * 128 : (t + 1) * 128, :], in_=o2f[:])
                o1T = transpose_to_bf16(o1, ident_bf, "1")
                o2T = transpose_to_bf16(o2, ident_bf, "2")
                zps = ps.tile([128, 128], F32, tag="z", bufs=2)
                nc.tensor.matmul(out=zps[:], lhsT=w11Tb[:], rhs=o1T[:], start=True, stop=False)
                nc.tensor.matmul(out=zps[:], lhsT=w12Tb[:], rhs=o2T[:], start=False, stop=True)
                GT = sb.tile([128, 128], BF16, tag="GT", bufs=2)
                nc.scalar.activation(
                    out=GT[:],
                    in_=zps[:],
                    func=mybir.ActivationFunctionType.Sigmoid,
                    bias=b1c[:, :1],
                )
                dT = sb.tile([128, 128], BF16, tag="dT", bufs=2)
                nc.vector.tensor_tensor(
                    out=dT[:], in0=o1T[:], in1=o2T[:], op=mybir.AluOpType.subtract
                )
                pT = sb.tile([128, 128], BF16, tag="pT", bufs=2)
                nc.vector.tensor_tensor(
                    out=pT[:], in0=GT[:], in1=dT[:], op=mybir.AluOpType.mult
                )
                hT = sb.tile([128, 128], BF16, tag="hT", bufs=2)
                nc.vector.tensor_tensor(
                    out=hT[:], in0=pT[:], in1=o2T[:], op=mybir.AluOpType.add
                )
                if kdbg == "l2":
                    htf = sb.tile([128, 128], F32, tag="htf", bufs=2)
                    nc.vector.tensor_copy(out=htf[:], in_=hT[:])
                    nc.sync.dma_start(
                        out=dbg_ht[t * 128 : (t + 1) * 128, :], in_=htf[:]
                    )
                # stage own hT into the AllGather input slab for layer 2
                j = chunk_of(t)
                nc.sync.dma_start(
                    out=HTO[j][:, (t - c0s[j]) * 128 : (t - c0s[j] + 1) * 128],
                    in_=hT[:],
                )

        # hT AllGathers AFTER the agg loop: their data deps (HTO writes of the
        # last batch) transitively order them after every gather DMA drained —
        # a collective executing while SWDGE gathers are in flight corrupts
        # data (observed as flaky NaN / device faults).
        for j in range(len(sizes)):
            ag_chunk(j)

        # layer-2 dense: all tiles from the AllGathered hT (lhsT directly)
        for j in range(len(sizes)):
            nj = sizes[j]
            for r in range(w):
                g0 = reg_off_rows[j] // 128 + r * nj
                done = 0
                while done < nj:
                    nt = min(8, nj - done)
                    hs = sb.tile([128, nt * 128], BF16, tag="xa", bufs=3)
                    nc.sync.dma_start(
                        out=hs[:],
                        in_=HTF[j][r, :, done * 128 : (done + nt) * 128],
                    )
                    dense_slab(g0 + done, nt, hs, W2b, T2f, T2r)
                    done += nt

        if kdbg == "l2":
            dbg_t20 = nc.dram_tensor("dbgt20", [r_split, 128], BF16, kind="ExternalOutput")
            dbg_t21 = nc.dram_tensor(
                "dbgt21", [np_pad - r_split, 128], BF16, kind="ExternalOutput"
            )
            nc.sync.dma_start(out=dbg_t20[:], in_=T2f[0][:])
            nc.sync.dma_start(out=dbg_t21[:], in_=T2f[1][:])

        # ---- layer 2 agg + gate + output ---------------------------------
        pending = prep_batch(0, (T2f, T2r), "2")
        for g in range(nb):
            fire_batch()
            msgs = pending
            if g + 1 < nb:
                pending = prep_batch(g + 1, (T2f, T2r), "2")
            kg = kg_of[g]
            for ti in range(kg):
                t = g * k + ti
                p1 = agg_tile(t, kg, ti, msgs["f"], dstf, dof, bc2r, F32, "f")
                p2 = agg_tile(t, kg, ti, msgs["r"], dstr, dor, bc2r, F32, "r")
                if dbg_p1 is not None:
                    nc.sync.dma_start(out=dbg_p1[t * 128 : (t + 1) * 128, :], in_=p1[:])
                    nc.sync.dma_start(out=dbg_p2[t * 128 : (t + 1) * 128, :], in_=p2[:])
                p1T = transpose_to_bf16(p1, ident_f32, "1")
                p2T = transpose_to_bf16(p2, ident_f32, "2")
                zps = ps.tile([128, 128], F32, tag="z", bufs=2)
                nc.tensor.matmul(out=zps[:], lhsT=w21Tb[:], rhs=p1T[:], start=True, stop=False)
                nc.tensor.matmul(out=zps[:], lhsT=w22Tb[:], rhs=p2T[:], start=False, stop=True)
                G2T = sb.tile([128, 128], BF16, tag="GT", bufs=2)
                nc.scalar.activation(
                    out=G2T[:],
                    in_=zps[:],
                    func=mybir.ActivationFunctionType.Sigmoid,
                    bias=b2c[:, :1],
                )
                g2p = ps.tile([128, 128], BF16, tag="tp", bufs=2)
                nc.tensor.transpose(out=g2p[:], in_=G2T[:], identity=ident_bf[:])
                g2s = sb.tile([128, 128], F32, tag="g2s", bufs=2)
                nc.vector.tensor_copy(out=g2s[:], in_=g2p[:])
                dd = sb.tile([128, 128], F32, tag="dd", bufs=2)
                nc.vector.tensor_tensor(
                    out=dd[:], in0=p1[:], in1=p2[:], op=mybir.AluOpType.subtract
                )
                pr = sb.tile([128, 128], F32, tag="pr", bufs=2)
                nc.vector.tensor_tensor(
                    out=pr[:], in0=dd[:], in1=g2s[:], op=mybir.AluOpType.mult
                )
                ot = sb.tile([128, 128], F32, tag="ot", bufs=2)
                nc.vector.tensor_tensor(
                    out=ot[:], in0=pr[:], in1=p2[:], op=mybir.AluOpType.add
                )
                nc.sync.dma_start(out=out_d[t * 128 : (t + 1) * 128, :], in_=ot[:])

    nc.compile()
    return nc


# ---------------------------------------------------------------------------
# full pipeline
# ---------------------------------------------------------------------------


def make_in_maps(inputs, meta, tables):
    import ml_dtypes

    w = W_CORES
    sh_real, sh_pad = meta["sh_real"], meta["sh_pad"]
    t_own, np_pad = meta["t_own"], meta["np_pad"]
    n_real = w * sh_real
    slot_of = meta["slot_of"]

    x = np.asarray(inputs["x"], np.float32)
    slots = slot_of(np.arange(n_real))
    x_slot = np.zeros((np_pad, D), np.float32)
    x_slot[slots] = x
    xT = np.ascontiguousarray(x_slot.T).astype(ml_dtypes.bfloat16)

    def t2(a):
        return np.ascontiguousarray(np.asarray(a, np.float32).T)

    W1 = np.asarray(inputs["W1"], np.float32)
    W2 = np.asarray(inputs["W2"], np.float32)
    b1c = np.asarray(inputs["b1"], np.float32).reshape(128, 1)
    b2c = np.asarray(inputs["b2"], np.float32).reshape(128, 1)
    bc1r = np.broadcast_to(np.asarray(inputs["bc1"], np.float32), (128, 128)).copy()
    bc2r = np.broadcast_to(np.asarray(inputs["bc2"], np.float32), (128, 128)).copy()

    in_maps = []
    for c in range(w):
        in_maps.append(
            dict(
                xT=xT,
                W1=W1, W2=W2,
                w11T=t2(inputs["w11"]), w12T=t2(inputs["w12"]),
                w21T=t2(inputs["w21"]), w22T=t2(inputs["w22"]),
                b1c=b1c, b2c=b2c, bc1r=bc1r, bc2r=bc2r,
                dof=tables["dof"][c], dor=tables["dor"][c],
                dallf=tables["dallf"], dallr=tables["dallr"],
                ilof=tables["ilo_f"][c], ihif=tables["ihi_f"][c],
                ilor=tables["ilo_r"][c], ihir=tables["ihi_r"][c],
                dstf=tables["dst_f"][c], dstr=tables["dst_r"][c],
            )
        )
    return in_maps


def assemble_output(results, meta):
    sh_real, sh_pad = meta["sh_real"], meta["sh_pad"]
    n_real = W_CORES * sh_real
    full = np.concatenate([r["out"] for r in results], axis=0)
    v = np.arange(n_real)
    rows = (v // sh_real) * sh_pad + (v % sh_real)
    return np.ascontiguousarray(full[rows]).astype(np.float32)


_CACHE = {}


def _get_program(meta, add_bc1, add_bc2):
    key = (meta["t_own"], meta["b_lo"], meta["b_hi"], meta["k"], add_bc1, add_bc2)
    if key not in _CACHE:
        _CACHE[key] = build_program(
            meta["t_own"], meta["b_lo"], meta["b_hi"], meta["k"],
            add_bc1=add_bc1, add_bc2=add_bc2,
        )
    return _CACHE[key]


def _install_ntff_hook():
    """Shim antenv.axon_hooks (absent in this image) so run_bass_kernel_spmd
    trace=True can capture NTFF profiles via libaxon_pjrt.so ctypes calls."""
    import contextlib
    import ctypes
    import types

    if "antenv.axon_hooks" in sys.modules:
        return
    so_path = "/opt/axon/libaxon_pjrt.so"
    holder = {}
    m = types.ModuleType("antenv.axon_hooks")
    m.set_axon_ntff_profile_hook = lambda h: holder.__setitem__("h", h)
    m.get_axon_ntff_profile_hook = lambda: holder.get("h")
    sys.modules["antenv.axon_hooks"] = m
    try:
        import antenv

        antenv.axon_hooks = m
    except ImportError:
        pass
    try:
        lib = ctypes.CDLL(so_path)
    except OSError:
        return
    if not hasattr(lib, "axon_start_nrt_profile"):
        return
    lib.axon_start_nrt_profile.argtypes = [
        ctypes.POINTER(ctypes.c_int64),
        ctypes.c_size_t,
    ]
    lib.axon_start_nrt_profile.restype = ctypes.c_int64
    lib.axon_stop_nrt_profile.argtypes = [ctypes.c_char_p]
    lib.axon_stop_nrt_profile.restype = ctypes.c_int64

    @contextlib.contextmanager
    def _hook(output_dir, device_ids):
        import jax

        jax.devices()
        if device_ids:
            ids = (ctypes.c_int64 * len(device_ids))(*device_ids)
            rc = lib.axon_start_nrt_profile(ids, len(device_ids))
        else:
            rc = lib.axon_start_nrt_profile(None, 0)
        if rc != 0:
            raise RuntimeError(f"axon_start_nrt_profile rc={rc}")
        try:
            yield
        finally:
            n = lib.axon_stop_nrt_profile(str(output_dir).encode())
            print(f"profile: {n} file(s) written to {output_dir}", file=sys.stderr)

    holder["h"] = _hook


def _patch_upload_artifacts():
    import concourse.bass_utils as bu

    bu.upload_artifacts = lambda tmpdir: tmpdir


def kernel(**inputs):
    x = np.asarray(inputs["x"], np.float32)
    n_real = x.shape[0]
    meta, tables = host_prepare(x, np.asarray(inputs["edge_index"]), n_real)
    add_bc1 = bool(np.any(np.asarray(inputs["bc1"]) != 0))
    add_bc2 = bool(np.any(np.asarray(inputs["bc2"]) != 0))
    nc = _get_program(meta, add_bc1, add_bc2)
    in_maps = make_in_maps(inputs, meta, tables)
    if bool(int(os.environ.get("KERNEL_TRACE", "0"))):
        _install_ntff_hook()
        _patch_upload_artifacts()
    res = run_bass_kernel_spmd(
        nc,
        in_maps,
        core_ids=list(range(W_CORES)),
        trace=bool(int(os.environ.get("KERNEL_TRACE", "0"))),
    )
    global LAST_EXEC_NS
    LAST_EXEC_NS = res.exec_time_ns
    if res.exec_time_ns is not None:
        print(f"HW exec time: {res.exec_time_ns} ns")
    return assemble_output(res.results, meta)


LAST_EXEC_NS = None


# revision 32
# speedup vs baseline: 1.6556x; 1.2682x over previous
"""Trainium2 Bass kernel for a 2-layer dual-direction gated GCN (DGGCN).

Contract: kernel(**inputs) takes the FULL unsharded inputs (as produced by
setup_inputs) and returns the FULL [N, D] float32 output.

Strategy (8 NeuronCores, node partition):
  - Nodes are remapped into a chunk-major padded slot space: the node axis is
    split into ag_chunks regions; within a region, cores' tiles are laid out
    contiguously per core so a chunked AllGather writes each region with one
    contiguous collective. Self-loops are explicit (v, v) edges. Edges are
    bucketed by destination tile (forward) / source tile (reverse) and split
    per tile into lo/hi halves of the slot space (region boundary) so gather
    indices fit dma_gather's int16 format. Uniform B_LO/B_HI block counts per
    tile keep the program SPMD.
  - Per layer each core computes h @ W for OWN tiles only, writes its slab,
    and a chunked AllGather replicates the bf16 message table to all cores.
    dinv[src] is NOT folded into the table; instead it is folded into the
    one-hot scatter matrices, so one table serves both edge directions.
  - Aggregation per own dst tile: batched dma_gather of table rows. All
    gathers are issued as prepare_only descriptor-generation (GPSIMD) one
    batch ahead and fired with trigger_dma, so SWDGE generation overlaps
    compute/DMA. Scatter matrices are built on-chip with a fused
    tensor_scalar (iota == dst_code) * dinv_src, then PE matmuls accumulate
    segment sums in PSUM; relu(x)*dinv_dst is fused on ACT.
"""

import os
import sys

sys.path.insert(0, "/opt/trn_rl_repo")

import numpy as np

import concourse.bacc as bacc
import concourse.bass as bass
import concourse.tile as tile
from concourse import mybir
from concourse.bass_utils import run_bass_kernel_spmd
from concourse.masks import make_identity

F32 = mybir.dt.float32
BF16 = mybir.dt.bfloat16
I32 = mybir.dt.int32
I16 = mybir.dt.int16

W_CORES = 8
D = 128
PAD_DST = 200.0  # sentinel local-dst id (never matches iota 0..127)
AG_CHUNKS = 2


# ---------------------------------------------------------------------------
# host-side graph preprocessing (index bucketing / sharding metadata only)
# ---------------------------------------------------------------------------


def _chunk_sizes(t_own):
    sizes = []
    base = 0
    for j in range(AG_CHUNKS):
        n = (t_own - base) // (AG_CHUNKS - j)
        if n > 0:
            sizes.append(n)
            base += n
    return sizes


def _pack16(flat):
    """Pack an int array [n] into dma_gather's [128, n//16] int16 layout:
    index i lives at partition i%16, column i//16, replicated to 8 stripes."""
    n = flat.shape[0]
    assert n % 16 == 0
    return np.tile(flat.reshape(n // 16, 16).T, (8, 1))


def host_prepare(x, edge_index, n_real):
    w = W_CORES
    assert n_real % w == 0
    sh_real = n_real // w
    t_own = (sh_real + 127) // 128
    sh_pad = t_own * 128
    t_all = w * t_own
    np_pad = t_all * 128
    sizes = _chunk_sizes(t_own)
    # chunk-major region starts (rows) and per-core tile starts within chunks
    reg_rows = [w * n * 128 for n in sizes]
    reg_off = np.concatenate([[0], np.cumsum(reg_rows)]).astype(np.int64)
    r_split = int(reg_off[1])  # lo/hi gather split at region-0 boundary
    assert r_split <= 32767 + 1 and (np_pad - r_split) <= 32767 + 1
    chunk_of_tile = np.concatenate(
        [np.full(n, j, np.int64) for j, n in enumerate(sizes)]
    )
    tile_in_chunk = np.concatenate([np.arange(n) for n in sizes])
    c0_of_chunk = np.concatenate([[0], np.cumsum(sizes)]).astype(np.int64)

    sizes_arr = np.asarray(sizes, np.int64)

    def slot_of(v):
        c = v // sh_real
        l = v % sh_real
        t = l // 128
        off = l % 128
        j = chunk_of_tile[t]
        return reg_off[j] + (c * sizes_arr[j] + tile_in_chunk[t]) * 128 + off

    src = np.asarray(edge_index[0], dtype=np.int64)
    dst = np.asarray(edge_index[1], dtype=np.int64)
    ss = slot_of(src)
    ds = slot_of(dst)
    selfs = slot_of(np.arange(n_real, dtype=np.int64))

    # degrees (+1 self-loop) in slot space; dinv on host
    deg_f = np.ones(np_pad, np.float32)
    deg_r = np.ones(np_pad, np.float32)
    np.add.at(deg_f, ds, 1.0)
    np.add.at(deg_r, ss, 1.0)
    dinv_f = (1.0 / np.sqrt(deg_f)).astype(np.float32)
    dinv_r = (1.0 / np.sqrt(deg_r)).astype(np.float32)

    # global tile id of a slot (for bucketing): region-aware
    def tile_of_slot(s):
        out = np.empty(s.shape, np.int64)
        for j in range(len(sizes)):
            m = (s >= reg_off[j]) & (s < reg_off[j + 1])
            rel = (s[m] - reg_off[j]) // 128  # core-major within region
            c = rel // sizes[j]
            t = rel % sizes[j]
            out[m] = c * t_own + c0_of_chunk[j] + t
        return out

    # self-loops as explicit edges
    agg_f = np.concatenate([ds, selfs])
    gat_f = np.concatenate([ss, selfs])
    agg_r = np.concatenate([ss, selfs])
    gat_r = np.concatenate([ds, selfs])

    def bucket(agg_slot, gather_slot):
        tile_id = tile_of_slot(agg_slot)
        hi = (gather_slot >= r_split).astype(np.int64)
        order = np.lexsort((hi, tile_id))
        t_s = tile_id[order]
        g_s = gather_slot[order]
        h_s = hi[order]
        loc_s = (agg_slot[order] % 128).astype(np.float32)
        n_lo = np.bincount(t_s[h_s == 0], minlength=t_all)
        n_hi = np.bincount(t_s[h_s == 1], minlength=t_all)
        return t_s, g_s, h_s, loc_s, n_lo, n_hi

    bf = bucket(agg_f, gat_f)
    br = bucket(agg_r, gat_r)
    b_lo = int(max(bf[4].max(), br[4].max()) + 127) // 128
    b_hi = int(max(bf[5].max(), br[5].max()) + 127) // 128
    bt = b_lo + b_hi

    def build_tables(t_s, g_s, h_s, loc_s, n_lo, n_hi):
        idx_lo = np.zeros((t_all, b_lo * 128), np.int16)
        idx_hi = np.zeros((t_all, b_hi * 128), np.int16)
        dst_t = np.full((t_all, bt * 128), PAD_DST, np.float32)
        n_edges = len(t_s)
        tile_starts = np.zeros(t_all + 1, np.int64)
        np.cumsum(n_lo + n_hi, out=tile_starts[1:])
        pos_in_tile = np.arange(n_edges) - tile_starts[t_s]
        pos_lo = pos_in_tile
        pos_hi = pos_in_tile - n_lo[t_s]
        mlo = h_s == 0
        mhi = h_s == 1
        idx_lo[t_s[mlo], pos_lo[mlo]] = g_s[mlo].astype(np.int16)
        idx_hi[t_s[mhi], pos_hi[mhi]] = (g_s[mhi] - r_split).astype(np.int16)
        dst_t[t_s[mlo], pos_lo[mlo]] = loc_s[mlo]
        dst_t[t_s[mhi], b_lo * 128 + pos_hi[mhi]] = loc_s[mhi]
        return idx_lo, idx_hi, dst_t

    tbl_f = build_tables(*bf)
    tbl_r = build_tables(*br)

    k = min(2, t_own)
    nb = (t_own + k - 1) // k

    def per_core(idx_lo, idx_hi, dst_t):
        import ml_dtypes
        idxlo_l, idxhi_l, dst_l = [], [], []
        for c in range(w):
            sl = slice(c * t_own, (c + 1) * t_own)
            ilo = idx_lo[sl]
            ihi = idx_hi[sl]
            plo = np.concatenate(
                [_pack16(ilo[g * k : min((g + 1) * k, t_own)].reshape(-1)) for g in range(nb)],
                axis=1,
            )
            phi = np.concatenate(
                [_pack16(ihi[g * k : min((g + 1) * k, t_own)].reshape(-1)) for g in range(nb)],
                axis=1,
            )

            def t_pack(a):
                return np.ascontiguousarray(
                    a.reshape(t_own, bt, 128).transpose(2, 0, 1).reshape(128, t_own * bt)
                ).astype(ml_dtypes.bfloat16)

            idxlo_l.append(np.ascontiguousarray(plo))
            idxhi_l.append(np.ascontiguousarray(phi))
            dst_l.append(t_pack(dst_t[sl]))
        return idxlo_l, idxhi_l, dst_l

    ilo_f, ihi_f, dst_f = per_core(*tbl_f)
    ilo_r, ihi_r, dst_r = per_core(*tbl_r)

    # own-tile dinv columns [128, t_own] per core per direction
    def own_dinv(dinv):
        cols = []
        for c in range(w):
            rows = np.empty(sh_pad, np.float32)
            for t in range(t_own):
                j = chunk_of_tile[t]
                r0 = reg_off[j] + (c * sizes[j] + tile_in_chunk[t]) * 128
                rows[t * 128 : (t + 1) * 128] = dinv[r0 : r0 + 128]
            cols.append(np.ascontiguousarray(rows.reshape(t_own, 128).T))
        return cols

    dof = own_dinv(dinv_f)
    dor = own_dinv(dinv_r)

    meta = dict(
        sh_real=sh_real, sh_pad=sh_pad, t_own=t_own, t_all=t_all, np_pad=np_pad,
        b_lo=b_lo, b_hi=b_hi, k=k, nb=nb, sizes=sizes, r_split=r_split,
        slot_of=slot_of,
    )
    tables = dict(
        ilo_f=ilo_f, ihi_f=ihi_f, dst_f=dst_f,
        ilo_r=ilo_r, ihi_r=ihi_r, dst_r=dst_r,
        dof=dof, dor=dor,
        dallf=np.ascontiguousarray(dinv_f.reshape(t_all, 128).T),
        dallr=np.ascontiguousarray(dinv_r.reshape(t_all, 128).T),
    )
    return meta, tables


# ---------------------------------------------------------------------------
# device program
# ---------------------------------------------------------------------------


def build_program(t_own, b_lo, b_hi, k, add_bc1=False, add_bc2=False):
    w = W_CORES
    t_all = w * t_own
    np_pad = t_all * 128
    sh_pad = t_own * 128
    bt = b_lo + b_hi
    nb = (t_own + k - 1) // k
    kg_of = [min(k, t_own - g * k) for g in range(nb)]
    lo_off = [sum(kg_of[:g]) * b_lo * 8 for g in range(nb + 1)]
    hi_off = [sum(kg_of[:g]) * b_hi * 8 for g in range(nb + 1)]
    lo_cols = lo_off[-1]
    hi_cols = hi_off[-1]
    sizes = _chunk_sizes(t_own)
    c0s = [0]
    for n in sizes:
        c0s.append(c0s[-1] + n)
    reg_off_rows = [0]
    for n in sizes:
        reg_off_rows.append(reg_off_rows[-1] + w * n * 128)
    r_split = reg_off_rows[1]
    assert len(sizes) == 2  # lo/hi gather tables == the two AG chunk tensors

    nc = bacc.Bacc(
        "TRN2", target_bir_lowering=False, debug=False, num_devices=w,
        num_swdge_queues=4, dynamic_dma_scratch_size=49152,
    )

    # ---- external I/O -----------------------------------------------------
    xT_d = nc.dram_tensor("xT", [128, np_pad], BF16, kind="ExternalInput")
    W1_d = nc.dram_tensor("W1", [128, 128], F32, kind="ExternalInput")
    W2_d = nc.dram_tensor("W2", [128, 128], F32, kind="ExternalInput")
    w11T_d = nc.dram_tensor("w11T", [128, 128], F32, kind="ExternalInput")
    w12T_d = nc.dram_tensor("w12T", [128, 128], F32, kind="ExternalInput")
    w21T_d = nc.dram_tensor("w21T", [128, 128], F32, kind="ExternalInput")
    w22T_d = nc.dram_tensor("w22T", [128, 128], F32, kind="ExternalInput")
    b1c_d = nc.dram_tensor("b1c", [128, 1], F32, kind="ExternalInput")
    b2c_d = nc.dram_tensor("b2c", [128, 1], F32, kind="ExternalInput")
    bc1r_d = nc.dram_tensor("bc1r", [128, 128], F32, kind="ExternalInput")
    bc2r_d = nc.dram_tensor("bc2r", [128, 128], F32, kind="ExternalInput")
    dof_d = nc.dram_tensor("dof", [128, t_own], F32, kind="ExternalInput")
    dor_d = nc.dram_tensor("dor", [128, t_own], F32, kind="ExternalInput")
    dallf_d = nc.dram_tensor("dallf", [128, t_all], F32, kind="ExternalInput")
    dallr_d = nc.dram_tensor("dallr", [128, t_all], F32, kind="ExternalInput")
    ilof_d = nc.dram_tensor("ilof", [128, lo_cols], I16, kind="ExternalInput")
    ihif_d = nc.dram_tensor("ihif", [128, hi_cols], I16, kind="ExternalInput")
    ilor_d = nc.dram_tensor("ilor", [128, lo_cols], I16, kind="ExternalInput")
    ihir_d = nc.dram_tensor("ihir", [128, hi_cols], I16, kind="ExternalInput")
    dstf_d = nc.dram_tensor("dstf", [128, t_own * bt], BF16, kind="ExternalInput")
    dstr_d = nc.dram_tensor("dstr", [128, t_own * bt], BF16, kind="ExternalInput")
    out_d = nc.dram_tensor("out", [sh_pad, 128], F32, kind="ExternalOutput")

    from contextlib import ExitStack

    with tile.TileContext(nc) as tc, ExitStack() as ctx:
        sb = ctx.enter_context(tc.tile_pool(name="sb", bufs=1))
        ps = ctx.enter_context(tc.tile_pool(name="ps", bufs=1, space="PSUM"))
        dr = ctx.enter_context(tc.tile_pool(name="dr", bufs=1, space="DRAM"))

        # message tables: per (layer, direction) pre-scaled by dinv_dir[u],
        # computed REPLICATED on every core (no collective). Separate lo/hi
        # tensors so lo gathers only depend on the first half of the dense
        # pass (subrange deps are tensor-granular).
        def table(name):
            return (
                dr.tile([r_split, 128], BF16, name=name + "lo"),
                dr.tile([np_pad - r_split, 128], BF16, name=name + "hi"),
            )

        T1f, T1r = table("T1f"), table("T1r")
        T2f, T2r = table("T2f"), table("T2r")
        # gated layer-1 output hT: own chunk slabs -> AllGathered (only coll.)
        HTO = [
            dr.tile([128, n * 128], BF16, name=f"HTO{j}")
            for j, n in enumerate(sizes)
        ]
        HTF = [
            dr.tile([w, 128, n * 128], BF16, name=f"HTF{j}", addr_space="Shared")
            for j, n in enumerate(sizes)
        ]

        # ---- constants / small persistent SBUF ----
        def load_const(dram, shape, dtype=F32, name=None):
            t = sb.tile(shape, dtype, name=name or dram.name + "_sb")
            nc.sync.dma_start(out=t[:], in_=dram[:])
            return t

        def load_cast_bf16(dram, name):
            t32 = sb.tile([128, 128], F32, name=name + "_f32")
            nc.sync.dma_start(out=t32[:], in_=dram[:])
            t16 = sb.tile([128, 128], BF16, name=name)
            nc.vector.tensor_copy(out=t16[:], in_=t32[:])
            return t16

        W1b = load_cast_bf16(W1_d, "W1b")
        W2b = load_cast_bf16(W2_d, "W2b")
        w11Tb = load_cast_bf16(w11T_d, "w11Tb")
        w12Tb = load_cast_bf16(w12T_d, "w12Tb")
        w21Tb = load_cast_bf16(w21T_d, "w21Tb")
        w22Tb = load_cast_bf16(w22T_d, "w22Tb")
        b1c = load_const(b1c_d, [128, 1], name="b1c")
        b2c = load_const(b2c_d, [128, 1], name="b2c")
        bc1r = load_const(bc1r_d, [128, 128], name="bc1r") if add_bc1 else None
        bc2r = load_const(bc2r_d, [128, 128], name="bc2r") if add_bc2 else None
        dof = load_const(dof_d, [128, t_own], name="dof")
        dor = load_const(dor_d, [128, t_own], name="dor")
        dstf = load_const(dstf_d, [128, t_own * bt], BF16, "dstf_sb")
        dstr = load_const(dstr_d, [128, t_own * bt], BF16, "dstr_sb")
        dallf = load_const(dallf_d, [128, t_all], F32, "dallf_sb")
        dallr = load_const(dallr_d, [128, t_all], F32, "dallr_sb")

        iota_i = sb.tile([128, 128], I32, name="iota_i")
        nc.gpsimd.iota(iota_i[:], pattern=[[1, 128]], base=0, channel_multiplier=0)
        iota_bf = sb.tile([128, 128], BF16, name="iota_bf")
        nc.vector.tensor_copy(out=iota_bf[:], in_=iota_i[:])
        # contiguous bt-times-repeated iota, so the per-tile IS_EQ reads a
        # non-broadcast in0 (broadcast reads force the slow 1-elem/cycle path)
        iota_full = sb.tile([128, bt, 128], BF16, name="iota_full")
        nc.vector.tensor_copy(
            out=iota_full[:],
            in_=iota_bf[:]
            .rearrange("p (o d) -> p o d", o=1)
            .to_broadcast([128, bt, 128]),
        )

        ident_bf = sb.tile([128, 128], BF16, name="ident_bf")
        make_identity(nc, ident_bf[:])
        ident_f32 = sb.tile([128, 128], F32, name="ident_f32")
        make_identity(nc, ident_f32[:])

        # own tile t -> column offset in xT (chunk-major slot space)
        def own_col(t):
            for j in range(len(sizes)):
                if t < c0s[j + 1]:
                    # column index c is baked per-core via per-core input? No:
                    # xT is the FULL table; own columns depend on core id.
                    raise AssertionError
            raise AssertionError

        # NOTE: xT is full; each core must read its own columns, which depend
        # on the core id. SPMD programs are identical across cores, so we pass
        # the own x slab as a separate per-core input instead.
        # (own_col unused; kept for clarity)

        # ---- dense pass: replicated over ALL tiles, slab-based ------------
        half_t = r_split // 128

        def dense_slab(g0, nt, xs_bf, Wb, Tf, Tr):
            """xs_bf: [128 d, nt*128 n] lhsT slab for tiles g0..g0+nt; writes
            dinv-scaled rows of the forward/reverse tables (one DMA each)."""
            hfs = sb.tile([128, nt * 128], BF16, tag="hfs", bufs=3)
            hrs = sb.tile([128, nt * 128], BF16, tag="hrs", bufs=3)
            for i in range(nt):
                g = g0 + i
                ph = ps.tile([128, 128], F32, tag="phd", bufs=2)
                nc.tensor.matmul(
                    out=ph[:],
                    lhsT=xs_bf[:, i * 128 : (i + 1) * 128],
                    rhs=Wb[:],
                    start=True,
                    stop=True,
                )
                nc.vector.tensor_scalar(
                    out=hfs[:, i * 128 : (i + 1) * 128],
                    in0=ph[:],
                    scalar1=dallf[:, g : g + 1],
                    scalar2=None,
                    op0=mybir.AluOpType.mult,
                )
                nc.scalar.activation(
                    out=hrs[:, i * 128 : (i + 1) * 128],
                    in_=ph[:],
                    func=mybir.ActivationFunctionType.Copy,
                    scale=dallr[:, g : g + 1],
                )
            hx = 0 if g0 < half_t else 1
            r0 = (g0 - hx * half_t) * 128
            for tbl, slb in ((Tf[hx], hfs), (Tr[hx], hrs)):
                nc.sync.dma_start(
                    out=tbl[r0 : r0 + nt * 128, :].rearrange("(t p) d -> p t d", p=128),
                    in_=slb[:].rearrange("p (t d) -> p t d", t=nt),
                )

        def ag_chunk(j):
            nc.gpsimd.collective_compute(
                "AllGather",
                mybir.AluOpType.bypass,
                replica_groups=[list(range(w))],
                ins=[HTO[j].opt()],
                outs=[HTF[j].opt()],
            )

        # layer-1 dense: all tiles from the replicated x (slabs never cross
        # the lo/hi boundary: half_t % slab == 0 enforced by slab choice)
        slab = 1
        for cand in range(min(8, half_t), 0, -1):
            if half_t % cand == 0 and (t_all - half_t) % cand == 0:
                slab = cand
                break
        for g0 in range(0, t_all, slab):
            xs = sb.tile([128, slab * 128], BF16, tag="xa", bufs=3)
            nc.sync.dma_start(
                out=xs[:], in_=xT_d[:, g0 * 128 : (g0 + slab) * 128]
            )
            dense_slab(g0, slab, xs, W1b, T1f, T1r)

        kdbg = os.environ.get("KDBG", "")
        if kdbg == "t1":
            dbg0 = nc.dram_tensor("dbg0", [r_split, 128], BF16, kind="ExternalOutput")
            dbg1 = nc.dram_tensor(
                "dbg1", [np_pad - r_split, 128], BF16, kind="ExternalOutput"
            )
            nc.sync.dma_start(out=dbg0[:], in_=T1f[0][:])
            nc.sync.dma_start(out=dbg1[:], in_=T1f[1][:])

        # ---- gather machinery --------------------------------------------
        gq_sem = [nc.alloc_semaphore(f"gq{q}") for q in range(4)]

        idx_src = dict(f=(ilof_d, ihif_d), r=(ilor_d, ihir_d))

        use_prep = bool(int(os.environ.get("KPREP", "0")))

        def prep_batch(g, Tpair, tag):
            """Emit idx loads + gathers for batch g, both dirs.
            Returns dict dir -> msg tile."""
            kg = kg_of[g]
            msgs = {}
            for di, dname in enumerate("fr"):
                T = Tpair[di]
                lo_d, hi_d = idx_src[dname]
                msg = sb.tile([128, kg * bt, 128], BF16, tag=f"msg{dname}", bufs=3)
                ilo = sb.tile([128, kg * b_lo * 8], I16, tag=f"ilo{dname}", bufs=3)
                nc.sync.dma_start(out=ilo[:], in_=lo_d[:, lo_off[g] : lo_off[g + 1]])
                ihi = sb.tile([128, kg * b_hi * 8], I16, tag=f"ihi{dname}", bufs=3)
                nc.sync.dma_start(out=ihi[:], in_=hi_d[:, hi_off[g] : hi_off[g + 1]])
                q_lo = di * 2
                q_hi = di * 2 + 1
                pk = dict(prepare_only=True) if use_prep else {}
                nc.gpsimd.dma_gather(
                    out_ap=msg[:, : kg * b_lo, :],
                    in_ap=T[0][:],
                    idxs_ap=ilo[:],
                    num_idxs=kg * b_lo * 128,
                    num_idxs_reg=kg * b_lo * 128,
                    elem_size=128,
                    single_packet=False,
                    queue_num=q_lo,
                    sem=gq_sem[q_lo] if use_prep else None,
                    **pk,
                )
                nc.gpsimd.dma_gather(
                    out_ap=msg[:, kg * b_lo :, :],
                    in_ap=T[1][:],
                    idxs_ap=ihi[:],
                    num_idxs=kg * b_hi * 128,
                    num_idxs_reg=kg * b_hi * 128,
                    elem_size=128,
                    single_packet=False,
                    queue_num=q_hi,
                    sem=gq_sem[q_hi] if use_prep else None,
                    **pk,
                )
                msgs[dname] = msg
            return msgs

        def fire_batch():
            if use_prep:
                for q in range(4):
                    nc.gpsimd.trigger_dma(count=None, queue_num=q)

        def msg_block(msg, kg, ti, b):
            if b < b_lo:
                return msg[:, ti * b_lo + b, :]
            return msg[:, kg * b_lo + ti * b_hi + (b - b_lo), :]

        def agg_tile(t, kg, ti, msg, dst_sb, dinvo, bcr, out_dtype, tagsuf):
            # one-hot S[e, b, j] = (iota j == dst_code[e, b])
            S = sb.tile([128, bt, 128], BF16, tag="S" + tagsuf, bufs=3)
            nc.vector.tensor_tensor(
                out=S[:],
                in0=iota_full[:],
                in1=dst_sb[:, t * bt : (t + 1) * bt].to_broadcast([128, bt, 128]),
                op=mybir.AluOpType.is_equal,
            )
            agg = ps.tile([128, 128], F32, tag="agg", bufs=2)
            for b in range(bt):
                nc.tensor.matmul(
                    out=agg[:],
                    lhsT=S[:, b, :],
                    rhs=msg_block(msg, kg, ti, b),
                    start=(b == 0),
                    stop=(b == bt - 1),
                )
            if bcr is not None:
                s2 = sb.tile([128, 128], F32, tag="s2" + tagsuf, bufs=2)
                nc.vector.tensor_scalar(
                    out=s2[:],
                    in0=agg[:],
                    scalar1=dinvo[:, t : t + 1],
                    scalar2=None,
                    op0=mybir.AluOpType.mult,
                )
                s3 = sb.tile([128, 128], F32, tag="s3" + tagsuf, bufs=2)
                nc.vector.tensor_tensor(
                    out=s3[:], in0=s2[:], in1=bcr[:], op=mybir.AluOpType.add
                )
                od = sb.tile([128, 128], out_dtype, tag="od" + tagsuf, bufs=2)
                nc.scalar.activation(
                    out=od[:], in_=s3[:], func=mybir.ActivationFunctionType.Relu
                )
                return od
            od = sb.tile([128, 128], out_dtype, tag="od" + tagsuf, bufs=2)
            nc.scalar.activation(
                out=od[:],
                in_=agg[:],
                func=mybir.ActivationFunctionType.Relu,
                scale=dinvo[:, t : t + 1],
            )
            return od

        def transpose_to_bf16(src, ident, tagsuf):
            tp = ps.tile([128, 128], src.dtype, tag="tp", bufs=2)
            nc.tensor.transpose(out=tp[:], in_=src[:], identity=ident[:])
            oT = sb.tile([128, 128], BF16, tag="oT" + tagsuf, bufs=2)
            nc.vector.tensor_copy(out=oT[:], in_=tp[:])
            return oT

        def chunk_of(t):
            for j in range(len(sizes)):
                if t < c0s[j + 1]:
                    return j
            raise AssertionError

        # ---- layer 1 agg + gate + layer-2 dense ---------------------------
        dbg_o1 = dbg_o2 = dbg_ht = dbg_p1 = dbg_p2 = None
        if kdbg == "o1":
            dbg_o1 = nc.dram_tensor("dbgo1", [sh_pad, 128], F32, kind="ExternalOutput")
            dbg_o2 = nc.dram_tensor("dbgo2", [sh_pad, 128], F32, kind="ExternalOutput")
        if kdbg == "l2":
            dbg_ht = nc.dram_tensor("dbght", [sh_pad, 128], F32, kind="ExternalOutput")
            dbg_p1 = nc.dram_tensor("dbgp1", [sh_pad, 128], F32, kind="ExternalOutput")
            dbg_p2 = nc.dram_tensor("dbgp2", [sh_pad, 128], F32, kind="ExternalOutput")
        pending = prep_batch(0, (T1f, T1r), "1")
        for g in range(nb):
            fire_batch()
            msgs = pending
            # prep next batch; layer-2 batch 0 must wait until T2's AllGather
            # chunks are EMITTED (Tile deps are emission-ordered), so it is
            # primed after this loop instead.
            if g + 1 < nb:
                pending = prep_batch(g + 1, (T1f, T1r), "1")
            kg = kg_of[g]
            for ti in range(kg):
                t = g * k + ti
                o1 = agg_tile(t, kg, ti, msgs["f"], dstf, dof, bc1r, BF16, "f")
                o2 = agg_tile(t, kg, ti, msgs["r"], dstr, dor, bc1r, BF16, "r")
                if dbg_o1 is not None:
                    o1f = sb.tile([128, 128], F32, tag="o1f", bufs=2)
                    nc.vector.tensor_copy(out=o1f[:], in_=o1[:])
                    nc.sync.dma_start(out=dbg_o1[t * 128 : (t + 1) * 128, :], in_=o1f[:])
                    o2f = sb.tile([128, 128], F32, tag="o2f", bufs=2)
                    nc.vector.tensor_copy(out=o2f[:], in_=o2[:])
                    nc.sync.dma_start(out=dbg_o2[t * 128 : (t + 1) * 128, :], in_=o2f[:])
                o1T = transpose_to_bf16(o1, ident_bf, "1")
                o2T = transpose_to_bf16(o2, ident_bf, "2")
                zps = ps.tile([128, 128], F32, tag="z", bufs=2)
                nc.tensor.matmul(out=zps[:], lhsT=w11Tb[:], rhs=o1T[:], start=True, stop=False)
                nc.tensor.matmul(out=zps[:], lhsT=w12Tb[:], rhs=o2T[:], start=False, stop=True)
                GT = sb.tile([128, 128], BF16, tag="GT", bufs=2)
                nc.scalar.activation(
                    out=GT[:],
                    in_=zps[:],
                    func=mybir.ActivationFunctionType.Sigmoid,
                    bias=b1c[:, :1],
                )
                dT = sb.tile([128, 128], BF16, tag="dT", bufs=2)
                nc.vector.tensor_tensor(
                    out=dT[:], in0=o1T[:], in1=o2T[:], op=mybir.AluOpType.subtract
                )
                pT = sb.tile([128, 128], BF16, tag="pT", bufs=2)
                nc.vector.tensor_tensor(
                    out=pT[:], in0=GT[:], in1=dT[:], op=mybir.AluOpType.mult
                )
                hT = sb.tile([128, 128], BF16, tag="hT", bufs=2)
                nc.vector.tensor_tensor(
                    out=hT[:], in0=pT[:], in1=o2T[:], op=mybir.AluOpType.add
                )
                if kdbg == "l2":
                    htf = sb.tile([128, 128], F32, tag="htf", bufs=2)
                    nc.vector.tensor_copy(out=htf[:], in_=hT[:])
                    nc.sync.dma_start(
                        out=dbg_ht[t * 128 : (t + 1) * 128, :], in_=htf[:]
                    )
                # stage own hT into the AllGather input slab for layer 2
                j = chunk_of(t)
                nc.sync.dma_start(
                    out=HTO[j][:, (t - c0s[j]) * 128 : (t - c0s[j] + 1) * 128],
                    in_=hT[:],
                )

        # hT AllGathers AFTER the agg loop: their data deps (HTO writes of the
        # last batch) transitively order them after every gather DMA drained —
        # a collective executing while SWDGE gathers are in flight corrupts
        # data (observed as flaky NaN / device faults).
        for j in range(len(sizes)):
            ag_chunk(j)

        # layer-2 dense: all tiles from the AllGathered hT (lhsT directly)
        for j in range(len(sizes)):
            nj = sizes[j]
            for r in range(w):
                g0 = reg_off_rows[j] // 128 + r * nj
                done = 0
                while done < nj:
                    nt = min(8, nj - done)
                    hs = sb.tile([128, nt * 128], BF16, tag="xa", bufs=3)
                    nc.sync.dma_start(
                        out=hs[:],
                        in_=HTF[j][r, :, done * 128 : (done + nt) * 128],
                    )
                    dense_slab(g0 + done, nt, hs, W2b, T2f, T2r)
                    done += nt

        if kdbg == "l2":
            dbg_t20 = nc.dram_tensor("dbgt20", [r_split, 128], BF16, kind="ExternalOutput")
            dbg_t21 = nc.dram_tensor(
                "dbgt21", [np_pad - r_split, 128], BF16, kind="ExternalOutput"
            )
            nc.sync.dma_start(out=dbg_t20[:], in_=T2f[0][:])
            nc.sync.dma_start(out=dbg_t21[:], in_=T2f[1][:])

        # ---- layer 2 agg + gate + output ---------------------------------
        pending = prep_batch(0, (T2f, T2r), "2")
        for g in range(nb):
            fire_batch()
            msgs = pending
            if g + 1 < nb:
                pending = prep_batch(g + 1, (T2f, T2r), "2")
            kg = kg_of[g]
            for ti in range(kg):
                t = g * k + ti
                p1 = agg_tile(t, kg, ti, msgs["f"], dstf, dof, bc2r, F32, "f")
                p2 = agg_tile(t, kg, ti, msgs["r"], dstr, dor, bc2r, F32, "r")
                if dbg_p1 is not None:
                    nc.sync.dma_start(out=dbg_p1[t * 128 : (t + 1) * 128, :], in_=p1[:])
                    nc.sync.dma_start(out=dbg_p2[t * 128 : (t + 1) * 128, :], in_=p2[:])
                p1T = transpose_to_bf16(p1, ident_f32, "1")
                p2T = transpose_to_bf16(p2, ident_f32, "2")
                zps = ps.tile([128, 128], F32, tag="z", bufs=2)
                nc.tensor.matmul(out=zps[:], lhsT=w21Tb[:], rhs=p1T[:], start=True, stop=False)
                nc.tensor.matmul(out=zps[:], lhsT=w22Tb[:], rhs=p2T[:], start=False, stop=True)
                G2T = sb.tile([128, 128], BF16, tag="GT", bufs=2)
                nc.scalar.activation(
                    out=G2T[:],
                    in_=zps[:],
                    func=mybir.ActivationFunctionType.Sigmoid,
                    bias=b2c[:, :1],
                )
                g2p = ps.tile([128, 128], BF16, tag="tp", bufs=2)
                nc.tensor.transpose(out=g2p[:], in_=G2T[:], identity=ident_bf[:])
                g2s = sb.tile([128, 128], F32, tag="g2s", bufs=2)
                nc.vector.tensor_copy(out=g2s[:], in_=g2p[:])
                dd = sb.tile([128, 128], F32, tag="dd", bufs=2)
                nc.vector.tensor_tensor(
                    out=dd[:], in0=p1[:], in1=p2[:], op=mybir.AluOpType.subtract
                )
                pr = sb.tile([128, 128], F32, tag="pr", bufs=2)
                nc.vector.tensor_tensor(
                    out=pr[:], in0=dd[:], in1=g2s[:], op=mybir.AluOpType.mult
                )
                ot = sb.tile([128, 128], F32, tag="ot", bufs=2)
                nc.vector.tensor_tensor(
                    out=ot[:], in0=pr[:], in1=p2[:], op=mybir.AluOpType.add
                )
                nc.sync.dma_start(out=out_d[t * 128 : (t + 1) * 128, :], in_=ot[:])

    nc.compile()
    return nc


# ---------------------------------------------------------------------------
# full pipeline
# ---------------------------------------------------------------------------


def make_in_maps(inputs, meta, tables):
    import ml_dtypes

    w = W_CORES
    sh_real, sh_pad = meta["sh_real"], meta["sh_pad"]
    t_own, np_pad = meta["t_own"], meta["np_pad"]
    n_real = w * sh_real
    slot_of = meta["slot_of"]

    x = np.asarray(inputs["x"], np.float32)
    slots = slot_of(np.arange(n_real))
    x_slot = np.zeros((np_pad, D), np.float32)
    x_slot[slots] = x
    xT = np.ascontiguousarray(x_slot.T).astype(ml_dtypes.bfloat16)

    def t2(a):
        return np.ascontiguousarray(np.asarray(a, np.float32).T)

    W1 = np.asarray(inputs["W1"], np.float32)
    W2 = np.asarray(inputs["W2"], np.float32)
    b1c = np.asarray(inputs["b1"], np.float32).reshape(128, 1)
    b2c = np.asarray(inputs["b2"], np.float32).reshape(128, 1)
    bc1r = np.broadcast_to(np.asarray(inputs["bc1"], np.float32), (128, 128)).copy()
    bc2r = np.broadcast_to(np.asarray(inputs["bc2"], np.float32), (128, 128)).copy()

    in_maps = []
    for c in range(w):
        in_maps.append(
            dict(
                xT=xT,
                W1=W1, W2=W2,
                w11T=t2(inputs["w11"]), w12T=t2(inputs["w12"]),
                w21T=t2(inputs["w21"]), w22T=t2(inputs["w22"]),
                b1c=b1c, b2c=b2c, bc1r=bc1r, bc2r=bc2r,
                dof=tables["dof"][c], dor=tables["dor"][c],
                dallf=tables["dallf"], dallr=tables["dallr"],
                ilof=tables["ilo_f"][c], ihif=tables["ihi_f"][c],
                ilor=tables["ilo_r"][c], ihir=tables["ihi_r"][c],
                dstf=tables["dst_f"][c], dstr=tables["dst_r"][c],
            )
        )
    return in_maps


def assemble_output(results, meta):
    sh_real, sh_pad = meta["sh_real"], meta["sh_pad"]
    n_real = W_CORES * sh_real
    full = np.concatenate([r["out"] for r in results], axis=0)
    v = np.arange(n_real)
    rows = (v // sh_real) * sh_pad + (v % sh_real)
    return np.ascontiguousarray(full[rows]).astype(np.float32)


_CACHE = {}


def _get_program(meta, add_bc1, add_bc2):
    key = (meta["t_own"], meta["b_lo"], meta["b_hi"], meta["k"], add_bc1, add_bc2)
    if key not in _CACHE:
        _CACHE[key] = build_program(
            meta["t_own"], meta["b_lo"], meta["b_hi"], meta["k"],
            add_bc1=add_bc1, add_bc2=add_bc2,
        )
    return _CACHE[key]


def _install_ntff_hook():
    """Shim antenv.axon_hooks (absent in this image) so run_bass_kernel_spmd
    trace=True can capture NTFF profiles via libaxon_pjrt.so ctypes calls."""
    import contextlib
    import ctypes
    import types

    if "antenv.axon_hooks" in sys.modules:
        return
    so_path = "/opt/axon/libaxon_pjrt.so"
    holder = {}
    m = types.ModuleType("antenv.axon_hooks")
    m.set_axon_ntff_profile_hook = lambda h: holder.__setitem__("h", h)
    m.get_axon_ntff_profile_hook = lambda: holder.get("h")
    sys.modules["antenv.axon_hooks"] = m
    try:
        import antenv

        antenv.axon_hooks = m
    except ImportError:
        pass
    try:
        lib = ctypes.CDLL(so_path)
    except OSError:
        return
    if not hasattr(lib, "axon_start_nrt_profile"):
        return
    lib.axon_start_nrt_profile.argtypes = [
        ctypes.POINTER(ctypes.c_int64),
        ctypes.c_size_t,
    ]
    lib.axon_start_nrt_profile.restype = ctypes.c_int64
    lib.axon_stop_nrt_profile.argtypes = [ctypes.c_char_p]
    lib.axon_stop_nrt_profile.restype = ctypes.c_int64

    @contextlib.contextmanager
    def _hook(output_dir, device_ids):
        import jax

        jax.devices()
        if device_ids:
            ids = (ctypes.c_int64 * len(device_ids))(*device_ids)
            rc = lib.axon_start_nrt_profile(ids, len(device_ids))
        else:
            rc = lib.axon_start_nrt_profile(None, 0)
        if rc != 0:
            raise RuntimeError(f"axon_start_nrt_profile rc={rc}")
        try:
            yield
        finally:
            n = lib.axon_stop_nrt_profile(str(output_dir).encode())
            print(f"profile: {n} file(s) written to {output_dir}", file=sys.stderr)

    holder["h"] = _hook


def _patch_upload_artifacts():
    import concourse.bass_utils as bu

    bu.upload_artifacts = lambda tmpdir: tmpdir


def kernel(**inputs):
    x = np.asarray(inputs["x"], np.float32)
    n_real = x.shape[0]
    meta, tables = host_prepare(x, np.asarray(inputs["edge_index"]), n_real)
    add_bc1 = bool(np.any(np.asarray(inputs["bc1"]) != 0))
    add_bc2 = bool(np.any(np.asarray(inputs["bc2"]) != 0))
    nc = _get_program(meta, add_bc1, add_bc2)
    in_maps = make_in_maps(inputs, meta, tables)
    if bool(int(os.environ.get("KERNEL_TRACE", "0"))):
        _install_ntff_hook()
        _patch_upload_artifacts()
    res = run_bass_kernel_spmd(
        nc,
        in_maps,
        core_ids=list(range(W_CORES)),
        trace=bool(int(os.environ.get("KERNEL_TRACE", "0"))),
    )
    global LAST_EXEC_NS
    LAST_EXEC_NS = res.exec_time_ns
    if res.exec_time_ns is not None:
        print(f"HW exec time: {res.exec_time_ns} ns")
    return assemble_output(res.results, meta)


LAST_EXEC_NS = None
